# revision 7
# baseline (speedup 1.0000x reference)
"""GNN message-passing (NodeModel) Trainium2 kernel, 8-core SPMD.

reference:
    agg = segment_sum(edge_attr, src, N)            # [N, D] scatter-add
    h = concat([x, agg], 1)                          # [N, 2D]
    h = BN(SiLU(h @ W1 + b1)); h = BN(SiLU(h @ W2 + b2)); out = h @ W3 + b3
    (BN in training mode: batch stats over all N nodes)

Strategy:
  - Host: permute nodes into 8 cores x 49 bins x 128 slots with per-bin edge
    loads balanced (LPT bin packing), bucket edges by owner bin, pad each bin
    to a whole number of 128-edge tiles with a cross-core-uniform tile
    schedule. All activations live feature-major ([D, nodes]) on device.
  - Device per core: for each bin, stream its edge tiles; one-hot(srcrel)
    matmuls accumulate the segment-sum in PSUM ([D, 128nodes]); MLP runs
    feature-major so BN stats are per-partition row sums (activation
    accum_out), BN scale/shift are folded into the next layer's weights.
    Global BN stats via two tiny AllGathers (plus an early warmup AllGather
    that absorbs the first-collective sync cost under the edge stream).
  - Dummy node slots (176 pad) are excluded from BN stats exactly.
"""

import os
import sys

for _p in ("/opt/trn_rl_repo",):
    if _p not in sys.path:
        sys.path.append(_p)

import numpy as np
import ml_dtypes

import concourse.bacc as bacc
from concourse import mybir
from concourse.tile import TileContext
from concourse.bass_utils import run_bass_kernel_spmd

# problem constants (hardcoded per contract)
N_NODES = 50000
N_EDGES = 600000
D = 128
BN_EPS = 1e-5
P = 128
NCORES = 8
BINS = 49                      # bins (128 nodes each) per core
NODES_PER_CORE = BINS * P      # 6272
REAL_PER_CORE = N_NODES // NCORES   # 6250
DUMMY_PER_CORE = NODES_PER_CORE - REAL_PER_CORE  # 22
MLP_TILE = 512
MLP_TILES = 13                 # 12 x 512 + 1 x 128

F32 = mybir.dt.float32
BF16 = mybir.dt.bfloat16

EDGE_BF16 = True   # convert edge tiles to bf16 on device for full-rate matmuls

LAST_EXEC_NS = None
_CACHE = {}


# ---------------------------------------------------------------- host prep

def _partition_graph(src):
    """Assign nodes to (core, slot) with balanced per-bin edge loads.

    Returns node_core[N], node_slot[N] (slot in [0, 6272)), and the uniform
    tile schedule K_rank[49] (edge tiles per bin rank, same for all cores).
    Bin rank 48 is the dummy bin: real slots [6144:6250), dummies [6250:6272).
    """
    import heapq

    deg = np.bincount(src, minlength=N_NODES)
    order = np.argsort(-deg, kind="stable")  # high degree first

    # snake round-robin over cores -> balanced degree sums, exactly 6250 each
    node_core = np.empty(N_NODES, np.int32)
    blocks = order.reshape(-1, NCORES)  # 6250 rows
    fwd = np.arange(NCORES)
    for i in range(blocks.shape[0]):
        cores = fwd if (i % 2 == 0) else fwd[::-1]
        node_core[blocks[i]] = cores

    node_slot = np.empty(N_NODES, np.int32)
    loads_all = np.empty((NCORES, BINS), np.int64)
    for c in range(NCORES):
        nodes_c = order[node_core[order] == c]  # still degree-sorted desc
        # LPT: assign each node to the least-loaded bin with free slots
        slots_free = [P] * BINS
        slots_free[BINS - 1] = P - DUMMY_PER_CORE  # dummy bin: 106 real slots
        heap = [(0, b) for b in range(BINS)]
        heapq.heapify(heap)
        bin_of = np.empty(len(nodes_c), np.int32)
        slot_in = np.empty(len(nodes_c), np.int32)
        loads = [0] * BINS
        used = [0] * BINS
        degs = deg[nodes_c]
        for i in range(len(nodes_c)):
            while True:
                load, b = heapq.heappop(heap)
                if used[b] < slots_free[b]:
                    break
            bin_of[i] = b
            slot_in[i] = used[b]
            used[b] += 1
            loads[b] += int(degs[i])
            if used[b] < slots_free[b]:
                heapq.heappush(heap, (loads[b], b))
        # rank bins by load desc so the tile-count profile aligns across
        # cores; dummy bin pinned to rank 48
        rest = sorted(range(BINS - 1), key=lambda b: -loads[b])
        rank_of = np.empty(BINS, np.int32)
        for r, b in enumerate(rest):
            rank_of[b] = r
        rank_of[BINS - 1] = BINS - 1
        node_slot[nodes_c] = rank_of[bin_of] * P + slot_in
        for b in range(BINS):
            loads_all[c, rank_of[b]] = loads[b]

    k_rank = np.maximum(1, np.ceil(loads_all.max(axis=0) / P).astype(np.int64))
    return node_core, node_slot, k_rank


def _prepare(x, edge_index, edge_attr):
    src = np.asarray(edge_index[0])
    node_core, node_slot, k_rank = _partition_graph(src)
    tot_tiles = int(k_rank.sum())
    rows = tot_tiles * P  # padded edge rows per core
    tile_off = np.concatenate([[0], np.cumsum(k_rank)])  # per-rank tile offset

    # per-edge destination
    e_core = node_core[src]
    e_slot = node_slot[src]
    e_rank = e_slot // P
    e_srcrel = (e_slot % P).astype(np.float32)
    key = e_core.astype(np.int64) * BINS + e_rank
    order = np.argsort(key, kind="stable")
    key_s = key[order]
    # position within each (core, rank) group
    counts = np.bincount(key_s, minlength=NCORES * BINS)
    starts = np.concatenate([[0], np.cumsum(counts)])[:-1]
    pos = np.arange(N_EDGES, dtype=np.int64) - starts[key_s]
    dest_row = tile_off[key_s % BINS] * P + pos  # row within core buffer
    dest_core = (key_s // BINS).astype(np.int64)

    edge_dt = ml_dtypes.bfloat16 if EDGE_BF16 else np.float32
    edges_all = np.zeros((NCORES, rows, D), np.float32)
    edges_all[dest_core, dest_row] = np.asarray(edge_attr)[order]
    srcrel_all = np.zeros((NCORES, rows), np.float32)
    srcrel_all[dest_core, dest_row] = e_srcrel[order]
    # srcrel device layout: [128(e), tot_tiles]
    srcrel_dev = np.ascontiguousarray(
        srcrel_all.reshape(NCORES, tot_tiles, P).transpose(0, 2, 1)
    )

    xt_all = np.zeros((NCORES, D, NODES_PER_CORE), np.float32)
    xt_all[node_core, :, node_slot] = np.asarray(x)

    return {
        "node_core": node_core,
        "node_slot": node_slot,
        "k_rank": [int(k) for k in k_rank],
        "tot_tiles": tot_tiles,
        "edges": edges_all,
        "srcrel": srcrel_dev,
        "xt": xt_all,
    }


# ------------------------------------------------------------- device build

def _build(k_rank):
    tot_tiles = int(sum(k_rank))
    nc = bacc.Bacc("TRN2", debug=False, num_devices=NCORES)

    edges_d = nc.declare_dram_parameter("edges", [tot_tiles * P, D], F32, isOutput=False)
    srcrel_d = nc.declare_dram_parameter("srcrel", [P, tot_tiles], F32, isOutput=False)
    xt_d = nc.declare_dram_parameter("xt", [D, NODES_PER_CORE], F32, isOutput=False)
    w1a_d = nc.declare_dram_parameter("w1a", [D, D], BF16, isOutput=False)
    w1b_d = nc.declare_dram_parameter("w1b", [D, D], BF16, isOutput=False)
    w2_d = nc.declare_dram_parameter("w2", [D, D], BF16, isOutput=False)
    w3_d = nc.declare_dram_parameter("w3", [D, D], BF16, isOutput=False)
    b1_d = nc.declare_dram_parameter("b1", [D], F32, isOutput=False)
    b2_d = nc.declare_dram_parameter("b2", [D], F32, isOutput=False)
    b3_d = nc.declare_dram_parameter("b3", [D], F32, isOutput=False)
    g1_d = nc.declare_dram_parameter("g1", [D], F32, isOutput=False)
    g2_d = nc.declare_dram_parameter("g2", [D], F32, isOutput=False)
    be1_d = nc.declare_dram_parameter("be1", [D], F32, isOutput=False)
    be2_d = nc.declare_dram_parameter("be2", [D], F32, isOutput=False)
    out_d = nc.declare_dram_parameter("out", [D, NODES_PER_CORE], F32, isOutput=True)
    dbg_d = nc.declare_dram_parameter("dbg", [NCORES], F32, isOutput=True)

    # collective bounce buffers
    warm_in = nc.dram_tensor("warm_in", [1], F32)
    warm_out = nc.dram_tensor("warm_out", [NCORES], F32, addr_space="Shared")
    cc_in = [nc.dram_tensor(f"cc_in{i}", [2 * D], F32) for i in range(2)]
    cc_out = [
        nc.dram_tensor(f"cc_out{i}", [NCORES * 2 * D], F32, addr_space="Shared")
        for i in range(2)
    ]
    rg = [list(range(NCORES))]

    edt = BF16 if EDGE_BF16 else F32
    inv_n = 1.0 / float(N_NODES)

    with (
        TileContext(nc) as tc,
        tc.tile_pool(name="const", bufs=1) as cpool,
        tc.tile_pool(name="big", bufs=1) as big,
        tc.tile_pool(name="ebin", bufs=4) as ebin_pool,
        tc.tile_pool(name="work", bufs=4) as work,
        tc.tile_pool(name="psum", bufs=4, space="PSUM") as pp,
        tc.tile_pool(name="psum_mlp", bufs=2, space="PSUM") as pmlp,
    ):
        # ---- constants
        iota_i = cpool.tile([P, P], mybir.dt.int32)
        nc.gpsimd.iota(iota_i[:], pattern=[[1, P]], channel_multiplier=0)
        iota_c = cpool.tile([P, P], edt)
        nc.vector.tensor_copy(iota_c[:], iota_i[:])

        srcrel_sb = cpool.tile([P, tot_tiles], F32)
        nc.sync.dma_start(out=srcrel_sb[:], in_=srcrel_d[:])

        def load_col(dram, nm):
            t = cpool.tile([P, 1], F32, name=f"col_{nm}")
            nc.sync.dma_start(out=t[:], in_=dram[:, None])
            return t

        b1_sb, b2_sb, b3_sb = load_col(b1_d, "b1"), load_col(b2_d, "b2"), load_col(b3_d, "b3")
        g1_sb, g2_sb = load_col(g1_d, "g1"), load_col(g2_d, "g2")
        be1_sb, be2_sb = load_col(be1_d, "be1"), load_col(be2_d, "be2")

        def load_w(dram, nm):
            t = cpool.tile([P, D], BF16, name=f"w_{nm}")
            nc.sync.dma_start(out=t[:], in_=dram[:])
            return t

        w1a_sb = load_w(w1a_d, "w1a")
        w1b_sb = load_w(w1b_d, "w1b")
        w2_sb = load_w(w2_d, "w2")
        w3_sb = load_w(w3_d, "w3")

        # ---- early dummy AllGather to absorb first-collective sync cost
        zz = cpool.tile([1, 1], F32)
        nc.vector.memset(zz[:], 0.0)
        nc.sync.dma_start(out=warm_in[:], in_=zz[0, :])
        nc.gpsimd.collective_compute(
            "AllGather", mybir.AluOpType.bypass, replica_groups=rg,
            ins=[warm_in[:]], outs=[warm_out[:]],
        )
        nc.sync.dma_start(out=dbg_d[:], in_=warm_out[:])

        # ---- big activation buffers (feature-major)
        xt_bf = big.tile([P, NODES_PER_CORE], BF16)
        agg_bf = big.tile([P, NODES_PER_CORE], BF16)
        h1_bf = big.tile([P, NODES_PER_CORE], BF16)
        h2_bf = big.tile([P, NODES_PER_CORE], BF16)

        # per-layer stat partials: [128, tiles] (sum, sumsq)
        s_part = [cpool.tile([P, MLP_TILES], F32, name=f"s_part{i}") for i in range(2)]
        q_part = [cpool.tile([P, MLP_TILES], F32, name=f"q_part{i}") for i in range(2)]

        def mlp_tile_cols(j):
            c0 = j * MLP_TILE
            c1 = min(NODES_PER_CORE, c0 + MLP_TILE)
            return c0, c1

        def silu_layer(li, j, ps, h_out, b_sb):
            """SiLU(ps + b) -> h_out cols of tile j, plus sum/sumsq partials
            (dummy slots excluded on the last tile)."""
            c0, c1 = mlp_tile_cols(j)
            w = c1 - c0
            real_w = w if j < MLP_TILES - 1 else (w - DUMMY_PER_CORE)
            nc.scalar.activation(
                out=h_out[:, c0 : c0 + real_w], in_=ps[:, :real_w],
                func=mybir.ActivationFunctionType.Silu,
                bias=b_sb[:, :1], scale=1.0,
                accum_out=s_part[li][:, j : j + 1],
            )
            if real_w < w:
                nc.scalar.activation(
                    out=h_out[:, c0 + real_w : c1], in_=ps[:, real_w:w],
                    func=mybir.ActivationFunctionType.Silu,
                    bias=b_sb[:, :1], scale=1.0,
                )
            sq = work.tile([P, MLP_TILE], BF16, tag="sq")
            nc.scalar.activation(
                out=sq[:, :real_w], in_=h_out[:, c0 : c0 + real_w],
                func=mybir.ActivationFunctionType.Square,
                accum_out=q_part[li][:, j : j + 1],
            )

        # ---- edge phase with interleaved layer-1 MLP
        tile_idx = 0
        for g in range(BINS):
            kg = k_rank[g]
            if g % 4 == 0:
                # prefetch + convert the xT chunk this group of bins feeds
                j = g // 4
                c0, c1 = mlp_tile_cols(j)
                xchunk = work.tile([P, MLP_TILE], F32, tag="xchunk")
                nc.sync.dma_start(out=xchunk[:, : c1 - c0], in_=xt_d[:, c0:c1])
                nc.vector.tensor_copy(xt_bf[:, c0:c1], xchunk[:, : c1 - c0])

            ebin = ebin_pool.tile([P, kg, D], F32, tag="ebin")
            nc.sync.dma_start(
                out=ebin[:],
                in_=edges_d[tile_idx * P : (tile_idx + kg) * P, :].rearrange(
                    "(t e) f -> e t f", t=kg
                ),
            )
            if EDGE_BF16:
                ebin_c = ebin_pool.tile([P, kg, D], BF16, tag="ebin_bf")
                nc.scalar.activation(
                    out=ebin_c.rearrange("e t f -> e (t f)"),
                    in_=ebin.rearrange("e t f -> e (t f)"),
                    func=mybir.ActivationFunctionType.Copy,
                )
            else:
                ebin_c = ebin

            ps_agg = pp.tile([P, P], F32, tag="ps_agg")
            for t in range(kg):
                oh = work.tile([P, P], edt, tag="oh")
                nc.vector.tensor_scalar(
                    out=oh[:], in0=iota_c[:],
                    scalar1=srcrel_sb[:, tile_idx : tile_idx + 1], scalar2=None,
                    op0=mybir.AluOpType.is_equal,
                )
                nc.tensor.matmul(
                    out=ps_agg[:], lhsT=ebin_c[:, t, :], rhs=oh[:],
                    start=(t == 0), stop=(t == kg - 1),
                )
                tile_idx += 1
            nc.vector.tensor_copy(agg_bf[:, g * P : (g + 1) * P], ps_agg[:])

            if g % 4 == 3 or g == BINS - 1:
                j = g // 4
                c0, c1 = mlp_tile_cols(j)
                ps1 = pmlp.tile([P, MLP_TILE], F32, tag="ps_mlp")
                nc.tensor.matmul(
                    out=ps1[:, : c1 - c0], lhsT=w1a_sb[:], rhs=xt_bf[:, c0:c1],
                    start=True, stop=False,
                )
                nc.tensor.matmul(
                    out=ps1[:, : c1 - c0], lhsT=w1b_sb[:], rhs=agg_bf[:, c0:c1],
                    start=False, stop=True,
                )
                silu_layer(0, j, ps1, h1_bf, b1_sb)

        # ---- BN stats + weight folding helper
        def bn_fold(li, w_sb, g_sb, be_sb, b_next_sb):
            """AllGather (sum, sumsq), compute scale/shift, fold into w_sb
            (rows) producing (w_fold, b_fold)."""
            s_loc = cpool.tile([P, 1], F32, tag=f"s_loc{li}")
            q_loc = cpool.tile([P, 1], F32, tag=f"q_loc{li}")
            nc.vector.reduce_sum(out=s_loc[:], in_=s_part[li][:], axis=mybir.AxisListType.X)
            nc.vector.reduce_sum(out=q_loc[:], in_=q_part[li][:], axis=mybir.AxisListType.X)
            nc.sync.dma_start(out=cc_in[li][0:D], in_=s_loc[:, 0])
            nc.sync.dma_start(out=cc_in[li][D : 2 * D], in_=q_loc[:, 0])
            nc.gpsimd.collective_compute(
                "AllGather", mybir.AluOpType.bypass, replica_groups=rg,
                ins=[cc_in[li][:]], outs=[cc_out[li][:]],
            )
            gath = work.tile([P, 2, NCORES], F32, tag=f"gath{li}")
            cc_view = cc_out[li].rearrange("(r kf) -> kf r", kf=2 * D)
            nc.sync.dma_start(out=gath[:, 0, :], in_=cc_view[0:D, :])
            nc.sync.dma_start(out=gath[:, 1, :], in_=cc_view[D : 2 * D, :])
            mean = cpool.tile([P, 1], F32, tag=f"mean{li}")
            msq = cpool.tile([P, 1], F32, tag=f"msq{li}")
            nc.vector.reduce_sum(out=mean[:], in_=gath[:, 0, :], axis=mybir.AxisListType.X)
            nc.vector.tensor_scalar_mul(mean[:], mean[:], inv_n)
            nc.vector.reduce_sum(out=msq[:], in_=gath[:, 1, :], axis=mybir.AxisListType.X)
            nc.vector.tensor_scalar_mul(msq[:], msq[:], inv_n)
            var = cpool.tile([P, 1], F32, tag=f"var{li}")
            nc.vector.tensor_tensor(
                out=var[:], in0=mean[:], in1=mean[:], op=mybir.AluOpType.mult
            )
            nc.vector.tensor_tensor(
                out=var[:], in0=msq[:], in1=var[:], op=mybir.AluOpType.subtract
            )
            nc.vector.tensor_scalar_add(var[:], var[:], BN_EPS)
            std = cpool.tile([P, 1], F32, tag=f"std{li}")
            nc.scalar.sqrt(std[:], var[:])
            rstd = cpool.tile([P, 1], F32, tag=f"rstd{li}")
            nc.vector.reciprocal(rstd[:], std[:])
            scale = cpool.tile([P, 1], F32, tag=f"scale{li}")
            nc.vector.tensor_tensor(
                out=scale[:], in0=g_sb[:], in1=rstd[:], op=mybir.AluOpType.mult
            )
            shift = cpool.tile([P, 1], F32, tag=f"shift{li}")
            nc.vector.tensor_tensor(
                out=shift[:], in0=mean[:], in1=scale[:], op=mybir.AluOpType.mult
            )
            nc.vector.tensor_tensor(
                out=shift[:], in0=be_sb[:], in1=shift[:], op=mybir.AluOpType.subtract
            )
            w_fold = cpool.tile([P, D], BF16, tag=f"wf{li}")
            nc.vector.tensor_scalar(
                out=w_fold[:], in0=w_sb[:], scalar1=scale[:, :1], scalar2=None,
                op0=mybir.AluOpType.mult,
            )
            shift_bf = cpool.tile([P, 1], BF16, tag=f"shift_bf{li}")
            nc.vector.tensor_copy(shift_bf[:], shift[:])
            ps_b = pmlp.tile([P, MLP_TILE], F32, tag="ps_mlp")
            nc.tensor.matmul(out=ps_b[:, :1], lhsT=w_sb[:], rhs=shift_bf[:], start=True, stop=True)
            b_fold = cpool.tile([P, 1], F32, tag=f"bf{li}")
            nc.vector.tensor_tensor(
                out=b_fold[:], in0=ps_b[:, :1], in1=b_next_sb[:], op=mybir.AluOpType.add
            )
            return w_fold, b_fold

        # ---- layer 2
        w2f, b2f = bn_fold(0, w2_sb, g1_sb, be1_sb, b2_sb)
        for j in range(MLP_TILES):
            c0, c1 = mlp_tile_cols(j)
            ps2 = pmlp.tile([P, MLP_TILE], F32, tag="ps_mlp")
            nc.tensor.matmul(
                out=ps2[:, : c1 - c0], lhsT=w2f[:], rhs=h1_bf[:, c0:c1],
                start=True, stop=True,
            )
            silu_layer(1, j, ps2, h2_bf, b2f)

        # ---- layer 3
        w3f, b3f = bn_fold(1, w3_sb, g2_sb, be2_sb, b3_sb)
        for j in range(MLP_TILES):
            c0, c1 = mlp_tile_cols(j)
            ps3 = pmlp.tile([P, MLP_TILE], F32, tag="ps_mlp")
            nc.tensor.matmul(
                out=ps3[:, : c1 - c0], lhsT=w3f[:], rhs=h2_bf[:, c0:c1],
                start=True, stop=True,
            )
            o = work.tile([P, MLP_TILE], F32, tag="otile")
            nc.scalar.activation(
                out=o[:, : c1 - c0], in_=ps3[:, : c1 - c0],
                func=mybir.ActivationFunctionType.Identity,
                bias=b3f[:, :1], scale=1.0,
            )
            nc.sync.dma_start(out=out_d[:, c0:c1], in_=o[:, : c1 - c0])

    nc.finalize()
    return nc


# ------------------------------------------------------------------- driver

def kernel(x, edge_index, edge_attr, W1, b1, g1, be1, W2, b2, g2, be2, W3, b3):
    global LAST_EXEC_NS
    x = np.asarray(x)
    prep = _prepare(x, np.asarray(edge_index), np.asarray(edge_attr))
    key = tuple(prep["k_rank"])
    if key not in _CACHE:
        _CACHE[key] = _build(prep["k_rank"])
    nc = _CACHE[key]

    W1 = np.asarray(W1, np.float32)
    bf = ml_dtypes.bfloat16
    shared = {
        "w1a": np.ascontiguousarray(W1[:D]).astype(bf),
        "w1b": np.ascontiguousarray(W1[D:]).astype(bf),
        "w2": np.asarray(W2, np.float32).astype(bf),
        "w3": np.asarray(W3, np.float32).astype(bf),
        "b1": np.asarray(b1, np.float32), "b2": np.asarray(b2, np.float32),
        "b3": np.asarray(b3, np.float32),
        "g1": np.asarray(g1, np.float32), "g2": np.asarray(g2, np.float32),
        "be1": np.asarray(be1, np.float32), "be2": np.asarray(be2, np.float32),
    }
    in_maps = []
    for c in range(NCORES):
        m = dict(shared)
        m["edges"] = prep["edges"][c]
        m["srcrel"] = prep["srcrel"][c]
        m["xt"] = prep["xt"][c]
        in_maps.append(m)

    trace = bool(os.environ.get("KERNEL_TRACE"))
    res = run_bass_kernel_spmd(
        nc, in_maps, core_ids=list(range(NCORES)), trace=trace
    )
    LAST_EXEC_NS = res.exec_time_ns

    outs = np.stack([np.asarray(res.results[c]["out"]) for c in range(NCORES)])
    # [core, D, slot] -> [N, D]
    out = outs[prep["node_core"], :, prep["node_slot"]]
    return np.ascontiguousarray(out.astype(np.float32))


# revision 8
# speedup vs baseline: 1.3234x; 1.3234x over previous
"""GNN message-passing (NodeModel) Trainium2 kernel, 8-core SPMD.

reference:
    agg = segment_sum(edge_attr, src, N)            # [N, D] scatter-add
    h = concat([x, agg], 1)                          # [N, 2D]
    h = BN(SiLU(h @ W1 + b1)); h = BN(SiLU(h @ W2 + b2)); out = h @ W3 + b3
    (BN in training mode: batch stats over all N nodes)

Strategy:
  - Host: permute nodes into 8 cores x 49 bins x 128 slots with per-bin edge
    loads balanced (LPT bin packing), bucket edges by owner bin, pad each bin
    to a whole number of 128-edge tiles with a cross-core-uniform tile
    schedule. All activations live feature-major ([D, nodes]) on device.
  - Device per core: for each bin, stream its edge tiles; one-hot(srcrel)
    matmuls accumulate the segment-sum in PSUM ([D, 128nodes]); MLP runs
    feature-major so BN stats are per-partition row sums (activation
    accum_out), BN scale/shift are folded into the next layer's weights.
    Global BN stats via two tiny AllGathers (plus an early warmup AllGather
    that absorbs the first-collective sync cost under the edge stream).
  - Dummy node slots (176 pad) are excluded from BN stats exactly.
"""

import os
import sys

for _p in ("/opt/trn_rl_repo",):
    if _p not in sys.path:
        sys.path.append(_p)

import numpy as np
import ml_dtypes

import concourse.bacc as bacc
from concourse import mybir
from concourse.tile import TileContext
from concourse.bass_utils import run_bass_kernel_spmd

# problem constants (hardcoded per contract)
N_NODES = 50000
N_EDGES = 600000
D = 128
BN_EPS = 1e-5
P = 128
NCORES = 8
BINS = 49                      # bins (128 nodes each) per core
NODES_PER_CORE = BINS * P      # 6272
REAL_PER_CORE = N_NODES // NCORES   # 6250
DUMMY_PER_CORE = NODES_PER_CORE - REAL_PER_CORE  # 22
MLP_TILE = 512
MLP_TILES = 13                 # 12 x 512 + 1 x 128

F32 = mybir.dt.float32
BF16 = mybir.dt.bfloat16

EDGE_BF16 = True   # convert edge tiles to bf16 on device for full-rate matmuls

LAST_EXEC_NS = None
_CACHE = {}


# ---------------------------------------------------------------- host prep

def _partition_graph(src):
    """Assign nodes to (core, slot) with balanced per-bin edge loads.

    Returns node_core[N], node_slot[N] (slot in [0, 6272)), and the uniform
    tile schedule K_rank[49] (edge tiles per bin rank, same for all cores).
    Bin rank 48 is the dummy bin: real slots [6144:6250), dummies [6250:6272).
    """
    import heapq

    deg = np.bincount(src, minlength=N_NODES)
    order = np.argsort(-deg, kind="stable")  # high degree first

    # snake round-robin over cores -> balanced degree sums, exactly 6250 each
    node_core = np.empty(N_NODES, np.int32)
    blocks = order.reshape(-1, NCORES)  # 6250 rows
    fwd = np.arange(NCORES)
    for i in range(blocks.shape[0]):
        cores = fwd if (i % 2 == 0) else fwd[::-1]
        node_core[blocks[i]] = cores

    node_slot = np.empty(N_NODES, np.int32)
    loads_all = np.empty((NCORES, BINS), np.int64)
    for c in range(NCORES):
        nodes_c = order[node_core[order] == c]  # still degree-sorted desc
        # LPT: assign each node to the least-loaded bin with free slots
        slots_free = [P] * BINS
        slots_free[BINS - 1] = P - DUMMY_PER_CORE  # dummy bin: 106 real slots
        heap = [(0, b) for b in range(BINS)]
        heapq.heapify(heap)
        bin_of = np.empty(len(nodes_c), np.int32)
        slot_in = np.empty(len(nodes_c), np.int32)
        loads = [0] * BINS
        used = [0] * BINS
        degs = deg[nodes_c]
        for i in range(len(nodes_c)):
            while True:
                load, b = heapq.heappop(heap)
                if used[b] < slots_free[b]:
                    break
            bin_of[i] = b
            slot_in[i] = used[b]
            used[b] += 1
            loads[b] += int(degs[i])
            if used[b] < slots_free[b]:
                heapq.heappush(heap, (loads[b], b))
        # rank bins by load desc so the tile-count profile aligns across
        # cores; dummy bin pinned to rank 48
        rest = sorted(range(BINS - 1), key=lambda b: -loads[b])
        rank_of = np.empty(BINS, np.int32)
        for r, b in enumerate(rest):
            rank_of[b] = r
        rank_of[BINS - 1] = BINS - 1
        node_slot[nodes_c] = rank_of[bin_of] * P + slot_in
        for b in range(BINS):
            loads_all[c, rank_of[b]] = loads[b]

    k_rank = np.maximum(1, np.ceil(loads_all.max(axis=0) / P).astype(np.int64))
    return node_core, node_slot, k_rank


def _prepare(x, edge_index, edge_attr):
    src = np.asarray(edge_index[0])
    node_core, node_slot, k_rank = _partition_graph(src)
    tot_tiles = int(k_rank.sum())
    rows = tot_tiles * P  # padded edge rows per core
    tile_off = np.concatenate([[0], np.cumsum(k_rank)])  # per-rank tile offset

    # per-edge destination
    e_core = node_core[src]
    e_slot = node_slot[src]
    e_rank = e_slot // P
    e_srcrel = (e_slot % P).astype(np.float32)
    key = e_core.astype(np.int64) * BINS + e_rank
    order = np.argsort(key, kind="stable")
    key_s = key[order]
    # position within each (core, rank) group
    counts = np.bincount(key_s, minlength=NCORES * BINS)
    starts = np.concatenate([[0], np.cumsum(counts)])[:-1]
    pos = np.arange(N_EDGES, dtype=np.int64) - starts[key_s]
    rank_s = (key_s % BINS).astype(np.int64)
    kg_s = np.asarray(k_rank)[rank_s]
    t_s = pos // P
    e_s = pos % P
    # bin block is [e][t][f]-major: row = bin_row_base + e*K_g + t
    dest_row = tile_off[rank_s] * P + e_s * kg_s + t_s
    dest_core = (key_s // BINS).astype(np.int64)

    edge_dt = ml_dtypes.bfloat16 if EDGE_BF16 else np.float32
    edges_all = np.zeros((NCORES, rows, D), edge_dt)
    edges_all[dest_core, dest_row] = np.asarray(edge_attr)[order].astype(edge_dt)
    srcrel_all = np.zeros((NCORES, rows), np.float32)
    srcrel_dest_row = tile_off[rank_s] * P + t_s * P + e_s
    srcrel_all[dest_core, srcrel_dest_row] = e_srcrel[order]
    # srcrel device layout: [128(e), tot_tiles]
    srcrel_dev = np.ascontiguousarray(
        srcrel_all.reshape(NCORES, tot_tiles, P).transpose(0, 2, 1)
    )

    xt_all = np.zeros((NCORES, D, NODES_PER_CORE), ml_dtypes.bfloat16)
    xt_all[node_core, :, node_slot] = np.asarray(x).astype(ml_dtypes.bfloat16)

    return {
        "node_core": node_core,
        "node_slot": node_slot,
        "k_rank": [int(k) for k in k_rank],
        "tot_tiles": tot_tiles,
        "edges": edges_all,
        "srcrel": srcrel_dev,
        "xt": xt_all,
    }


# ------------------------------------------------------------- device build

def _build(k_rank):
    tot_tiles = int(sum(k_rank))
    nc = bacc.Bacc("TRN2", debug=False, num_devices=NCORES)

    edges_d = nc.declare_dram_parameter("edges", [tot_tiles * P, D], BF16, isOutput=False)
    srcrel_d = nc.declare_dram_parameter("srcrel", [P, tot_tiles], F32, isOutput=False)
    xt_d = nc.declare_dram_parameter("xt", [D, NODES_PER_CORE], BF16, isOutput=False)
    w1a_d = nc.declare_dram_parameter("w1a", [D, D], BF16, isOutput=False)
    w1b_d = nc.declare_dram_parameter("w1b", [D, D], BF16, isOutput=False)
    w2_d = nc.declare_dram_parameter("w2", [D, D], BF16, isOutput=False)
    w3_d = nc.declare_dram_parameter("w3", [D, D], BF16, isOutput=False)
    b1_d = nc.declare_dram_parameter("b1", [D], F32, isOutput=False)
    b2_d = nc.declare_dram_parameter("b2", [D], F32, isOutput=False)
    b3_d = nc.declare_dram_parameter("b3", [D], F32, isOutput=False)
    g1_d = nc.declare_dram_parameter("g1", [D], F32, isOutput=False)
    g2_d = nc.declare_dram_parameter("g2", [D], F32, isOutput=False)
    be1_d = nc.declare_dram_parameter("be1", [D], F32, isOutput=False)
    be2_d = nc.declare_dram_parameter("be2", [D], F32, isOutput=False)
    out_d = nc.declare_dram_parameter("out", [D, NODES_PER_CORE], F32, isOutput=True)
    dbg_d = nc.declare_dram_parameter("dbg", [NCORES], F32, isOutput=True)

    # collective bounce buffers
    warm_in = nc.dram_tensor("warm_in", [1], F32)
    warm_out = nc.dram_tensor("warm_out", [NCORES], F32, addr_space="Shared")
    cc_in = [nc.dram_tensor(f"cc_in{i}", [2 * D], F32) for i in range(2)]
    cc_out = [
        nc.dram_tensor(f"cc_out{i}", [NCORES * 2 * D], F32, addr_space="Shared")
        for i in range(2)
    ]
    rg = [list(range(NCORES))]

    edt = BF16 if EDGE_BF16 else F32
    inv_n = 1.0 / float(N_NODES)

    with (
        TileContext(nc) as tc,
        tc.tile_pool(name="const", bufs=1) as cpool,
        tc.tile_pool(name="big", bufs=1) as big,
        tc.tile_pool(name="ebin", bufs=6) as ebin_pool,
        tc.tile_pool(name="work", bufs=4) as work,
        tc.tile_pool(name="psum", bufs=4, space="PSUM") as pp,
        tc.tile_pool(name="psum_mlp", bufs=2, space="PSUM") as pmlp,
    ):
        # ---- constants
        iota_i = cpool.tile([P, P], mybir.dt.int32)
        nc.gpsimd.iota(iota_i[:], pattern=[[1, P]], channel_multiplier=0)
        iota_c = cpool.tile([P, P], edt)
        nc.vector.tensor_copy(iota_c[:], iota_i[:])

        srcrel_sb = cpool.tile([P, tot_tiles], F32)
        nc.sync.dma_start(out=srcrel_sb[:], in_=srcrel_d[:])

        def load_col(dram, nm):
            t = cpool.tile([P, 1], F32, name=f"col_{nm}")
            nc.sync.dma_start(out=t[:], in_=dram[:, None])
            return t

        b1_sb, b2_sb, b3_sb = load_col(b1_d, "b1"), load_col(b2_d, "b2"), load_col(b3_d, "b3")
        g1_sb, g2_sb = load_col(g1_d, "g1"), load_col(g2_d, "g2")
        be1_sb, be2_sb = load_col(be1_d, "be1"), load_col(be2_d, "be2")

        def load_w(dram, nm):
            t = cpool.tile([P, D], BF16, name=f"w_{nm}")
            nc.sync.dma_start(out=t[:], in_=dram[:])
            return t

        w1a_sb = load_w(w1a_d, "w1a")
        w1b_sb = load_w(w1b_d, "w1b")
        w2_sb = load_w(w2_d, "w2")
        w3_sb = load_w(w3_d, "w3")

        # ---- early dummy AllGather to absorb first-collective sync cost
        zz = cpool.tile([1, 1], F32)
        nc.vector.memset(zz[:], 0.0)
        nc.sync.dma_start(out=warm_in[:], in_=zz[0, :])
        nc.gpsimd.collective_compute(
            "AllGather", mybir.AluOpType.bypass, replica_groups=rg,
            ins=[warm_in[:]], outs=[warm_out[:]],
        )
        nc.sync.dma_start(out=dbg_d[:], in_=warm_out[:])

        # ---- big activation buffers (feature-major)
        xt_bf = big.tile([P, NODES_PER_CORE], BF16)
        nc.sync.dma_start(out=xt_bf[:], in_=xt_d[:])
        agg_bf = big.tile([P, NODES_PER_CORE], BF16)
        h1_bf = big.tile([P, NODES_PER_CORE], BF16)
        h2_bf = big.tile([P, NODES_PER_CORE], BF16)

        # per-layer stat partials: [128, tiles] (sum, sumsq)
        s_part = [cpool.tile([P, MLP_TILES], F32, name=f"s_part{i}") for i in range(2)]
        q_part = [cpool.tile([P, MLP_TILES], F32, name=f"q_part{i}") for i in range(2)]

        def mlp_tile_cols(j):
            c0 = j * MLP_TILE
            c1 = min(NODES_PER_CORE, c0 + MLP_TILE)
            return c0, c1

        def silu_layer(li, j, ps, h_out, b_sb):
            """SiLU(ps + b) -> h_out cols of tile j, plus sum/sumsq partials
            (dummy slots excluded on the last tile)."""
            c0, c1 = mlp_tile_cols(j)
            w = c1 - c0
            real_w = w if j < MLP_TILES - 1 else (w - DUMMY_PER_CORE)
            nc.scalar.activation(
                out=h_out[:, c0 : c0 + real_w], in_=ps[:, :real_w],
                func=mybir.ActivationFunctionType.Silu,
                bias=b_sb[:, :1], scale=1.0,
                accum_out=s_part[li][:, j : j + 1],
            )
            if real_w < w:
                nc.scalar.activation(
                    out=h_out[:, c0 + real_w : c1], in_=ps[:, real_w:w],
                    func=mybir.ActivationFunctionType.Silu,
                    bias=b_sb[:, :1], scale=1.0,
                )
            sq = work.tile([P, MLP_TILE], BF16, tag="sq")
            nc.scalar.activation(
                out=sq[:, :real_w], in_=h_out[:, c0 : c0 + real_w],
                func=mybir.ActivationFunctionType.Square,
                accum_out=q_part[li][:, j : j + 1],
            )

        # ---- edge phase with interleaved layer-1 MLP
        tile_idx = 0
        for g in range(BINS):
            kg = k_rank[g]
            ebin_c = ebin_pool.tile([P, kg, D], BF16, tag="ebin")
            nc.sync.dma_start(
                out=ebin_c.rearrange("e t f -> e (t f)"),
                in_=edges_d[tile_idx * P : (tile_idx + kg) * P, :].rearrange(
                    "(e q) f -> e (q f)", e=P
                ),
            )

            ps_agg = pp.tile([P, P], F32, tag="ps_agg")
            for t in range(kg):
                oh = work.tile([P, P], edt, tag="oh")
                nc.vector.tensor_scalar(
                    out=oh[:], in0=iota_c[:],
                    scalar1=srcrel_sb[:, tile_idx : tile_idx + 1], scalar2=None,
                    op0=mybir.AluOpType.is_equal,
                )
                nc.tensor.matmul(
                    out=ps_agg[:], lhsT=ebin_c[:, t, :], rhs=oh[:],
                    start=(t == 0), stop=(t == kg - 1),
                )
                tile_idx += 1
            nc.scalar.copy(agg_bf[:, g * P : (g + 1) * P], ps_agg[:])

            if g % 4 == 3 or g == BINS - 1:
                j = g // 4
                c0, c1 = mlp_tile_cols(j)
                ps1 = pmlp.tile([P, MLP_TILE], F32, tag="ps_mlp")
                nc.tensor.matmul(
                    out=ps1[:, : c1 - c0], lhsT=w1a_sb[:], rhs=xt_bf[:, c0:c1],
                    start=True, stop=False,
                )
                nc.tensor.matmul(
                    out=ps1[:, : c1 - c0], lhsT=w1b_sb[:], rhs=agg_bf[:, c0:c1],
                    start=False, stop=True,
                )
                silu_layer(0, j, ps1, h1_bf, b1_sb)

        # ---- BN stats + weight folding helper
        def bn_fold(li, w_sb, g_sb, be_sb, b_next_sb):
            """AllGather (sum, sumsq), compute scale/shift, fold into w_sb
            (rows) producing (w_fold, b_fold)."""
            s_loc = cpool.tile([P, 1], F32, tag=f"s_loc{li}")
            q_loc = cpool.tile([P, 1], F32, tag=f"q_loc{li}")
            nc.vector.reduce_sum(out=s_loc[:], in_=s_part[li][:], axis=mybir.AxisListType.X)
            nc.vector.reduce_sum(out=q_loc[:], in_=q_part[li][:], axis=mybir.AxisListType.X)
            nc.sync.dma_start(out=cc_in[li][0:D], in_=s_loc[:, 0])
            nc.sync.dma_start(out=cc_in[li][D : 2 * D], in_=q_loc[:, 0])
            nc.gpsimd.collective_compute(
                "AllGather", mybir.AluOpType.bypass, replica_groups=rg,
                ins=[cc_in[li][:]], outs=[cc_out[li][:]],
            )
            gath = work.tile([P, 2, NCORES], F32, tag=f"gath{li}")
            cc_view = cc_out[li].rearrange("(r kf) -> kf r", kf=2 * D)
            nc.sync.dma_start(out=gath[:, 0, :], in_=cc_view[0:D, :])
            nc.sync.dma_start(out=gath[:, 1, :], in_=cc_view[D : 2 * D, :])
            mean = cpool.tile([P, 1], F32, tag=f"mean{li}")
            msq = cpool.tile([P, 1], F32, tag=f"msq{li}")
            nc.vector.reduce_sum(out=mean[:], in_=gath[:, 0, :], axis=mybir.AxisListType.X)
            nc.vector.tensor_scalar_mul(mean[:], mean[:], inv_n)
            nc.vector.reduce_sum(out=msq[:], in_=gath[:, 1, :], axis=mybir.AxisListType.X)
            nc.vector.tensor_scalar_mul(msq[:], msq[:], inv_n)
            var = cpool.tile([P, 1], F32, tag=f"var{li}")
            nc.vector.tensor_tensor(
                out=var[:], in0=mean[:], in1=mean[:], op=mybir.AluOpType.mult
            )
            nc.vector.tensor_tensor(
                out=var[:], in0=msq[:], in1=var[:], op=mybir.AluOpType.subtract
            )
            nc.vector.tensor_scalar_add(var[:], var[:], BN_EPS)
            std = cpool.tile([P, 1], F32, tag=f"std{li}")
            nc.scalar.sqrt(std[:], var[:])
            rstd = cpool.tile([P, 1], F32, tag=f"rstd{li}")
            nc.vector.reciprocal(rstd[:], std[:])
            scale = cpool.tile([P, 1], F32, tag=f"scale{li}")
            nc.vector.tensor_tensor(
                out=scale[:], in0=g_sb[:], in1=rstd[:], op=mybir.AluOpType.mult
            )
            shift = cpool.tile([P, 1], F32, tag=f"shift{li}")
            nc.vector.tensor_tensor(
                out=shift[:], in0=mean[:], in1=scale[:], op=mybir.AluOpType.mult
            )
            nc.vector.tensor_tensor(
                out=shift[:], in0=be_sb[:], in1=shift[:], op=mybir.AluOpType.subtract
            )
            w_fold = cpool.tile([P, D], BF16, tag=f"wf{li}")
            nc.vector.tensor_scalar(
                out=w_fold[:], in0=w_sb[:], scalar1=scale[:, :1], scalar2=None,
                op0=mybir.AluOpType.mult,
            )
            shift_bf = cpool.tile([P, 1], BF16, tag=f"shift_bf{li}")
            nc.vector.tensor_copy(shift_bf[:], shift[:])
            ps_b = pmlp.tile([P, MLP_TILE], F32, tag="ps_mlp")
            nc.tensor.matmul(out=ps_b[:, :1], lhsT=w_sb[:], rhs=shift_bf[:], start=True, stop=True)
            b_fold = cpool.tile([P, 1], F32, tag=f"bf{li}")
            nc.vector.tensor_tensor(
                out=b_fold[:], in0=ps_b[:, :1], in1=b_next_sb[:], op=mybir.AluOpType.add
            )
            return w_fold, b_fold

        # ---- layer 2
        w2f, b2f = bn_fold(0, w2_sb, g1_sb, be1_sb, b2_sb)
        for j in range(MLP_TILES):
            c0, c1 = mlp_tile_cols(j)
            ps2 = pmlp.tile([P, MLP_TILE], F32, tag="ps_mlp")
            nc.tensor.matmul(
                out=ps2[:, : c1 - c0], lhsT=w2f[:], rhs=h1_bf[:, c0:c1],
                start=True, stop=True,
            )
            silu_layer(1, j, ps2, h2_bf, b2f)

        # ---- layer 3
        w3f, b3f = bn_fold(1, w3_sb, g2_sb, be2_sb, b3_sb)
        for j in range(MLP_TILES):
            c0, c1 = mlp_tile_cols(j)
            ps3 = pmlp.tile([P, MLP_TILE], F32, tag="ps_mlp")
            nc.tensor.matmul(
                out=ps3[:, : c1 - c0], lhsT=w3f[:], rhs=h2_bf[:, c0:c1],
                start=True, stop=True,
            )
            o = work.tile([P, MLP_TILE], F32, tag="otile")
            nc.scalar.activation(
                out=o[:, : c1 - c0], in_=ps3[:, : c1 - c0],
                func=mybir.ActivationFunctionType.Identity,
                bias=b3f[:, :1], scale=1.0,
            )
            nc.sync.dma_start(out=out_d[:, c0:c1], in_=o[:, : c1 - c0])

    nc.finalize()
    return nc


# ------------------------------------------------------------------- driver

def kernel(x, edge_index, edge_attr, W1, b1, g1, be1, W2, b2, g2, be2, W3, b3):
    global LAST_EXEC_NS
    x = np.asarray(x)
    prep = _prepare(x, np.asarray(edge_index), np.asarray(edge_attr))
    key = tuple(prep["k_rank"])
    if key not in _CACHE:
        _CACHE[key] = _build(prep["k_rank"])
    nc = _CACHE[key]

    W1 = np.asarray(W1, np.float32)
    bf = ml_dtypes.bfloat16
    shared = {
        "w1a": np.ascontiguousarray(W1[:D]).astype(bf),
        "w1b": np.ascontiguousarray(W1[D:]).astype(bf),
        "w2": np.asarray(W2, np.float32).astype(bf),
        "w3": np.asarray(W3, np.float32).astype(bf),
        "b1": np.asarray(b1, np.float32), "b2": np.asarray(b2, np.float32),
        "b3": np.asarray(b3, np.float32),
        "g1": np.asarray(g1, np.float32), "g2": np.asarray(g2, np.float32),
        "be1": np.asarray(be1, np.float32), "be2": np.asarray(be2, np.float32),
    }
    in_maps = []
    for c in range(NCORES):
        m = dict(shared)
        m["edges"] = prep["edges"][c]
        m["srcrel"] = prep["srcrel"][c]
        m["xt"] = prep["xt"][c]
        in_maps.append(m)

    trace = bool(os.environ.get("KERNEL_TRACE"))
    res = run_bass_kernel_spmd(
        nc, in_maps, core_ids=list(range(NCORES)), trace=trace
    )
    LAST_EXEC_NS = res.exec_time_ns

    outs = np.stack([np.asarray(res.results[c]["out"]) for c in range(NCORES)])
    # [core, D, slot] -> [N, D]
    out = outs[prep["node_core"], :, prep["node_slot"]]
    return np.ascontiguousarray(out.astype(np.float32))


# revision 10
# speedup vs baseline: 1.4960x; 1.1304x over previous
"""GNN message-passing (NodeModel) Trainium2 kernel, 8-core SPMD.

reference:
    agg = segment_sum(edge_attr, src, N)            # [N, D] scatter-add
    h = concat([x, agg], 1)                          # [N, 2D]
    h = BN(SiLU(h @ W1 + b1)); h = BN(SiLU(h @ W2 + b2)); out = h @ W3 + b3
    (BN in training mode: batch stats over all N nodes)

Strategy:
  - Host: permute nodes into 8 cores x 49 bins x 128 slots with per-bin edge
    loads balanced (LPT bin packing), bucket edges by owner bin, pad each bin
    to a whole number of 128-edge tiles with a cross-core-uniform tile
    schedule. All activations live feature-major ([D, nodes]) on device.
  - Device per core: for each bin, stream its edge tiles; one-hot(srcrel)
    matmuls accumulate the segment-sum in PSUM ([D, 128nodes]); MLP runs
    feature-major so BN stats are per-partition row sums (activation
    accum_out), BN scale/shift are folded into the next layer's weights.
    Global BN stats via two tiny AllGathers (plus an early warmup AllGather
    that absorbs the first-collective sync cost under the edge stream).
  - Dummy node slots (176 pad) are excluded from BN stats exactly.
"""

import os
import sys

for _p in ("/opt/trn_rl_repo",):
    if _p not in sys.path:
        sys.path.append(_p)

import numpy as np
import ml_dtypes

import concourse.bacc as bacc
from concourse import mybir
from concourse.tile import TileContext
from concourse.bass_utils import run_bass_kernel_spmd

# problem constants (hardcoded per contract)
N_NODES = 50000
N_EDGES = 600000
D = 128
BN_EPS = 1e-5
P = 128
NCORES = 8
BINS = 49                      # bins (128 nodes each) per core
NODES_PER_CORE = BINS * P      # 6272
REAL_PER_CORE = N_NODES // NCORES   # 6250
DUMMY_PER_CORE = NODES_PER_CORE - REAL_PER_CORE  # 22
MLP_TILE = 512
MLP_TILES = 13                 # 12 x 512 + 1 x 128
OH_CHUNK = 6                   # onehot tiles generated per DVE op

F32 = mybir.dt.float32
BF16 = mybir.dt.bfloat16

EDGE_BF16 = True   # convert edge tiles to bf16 on device for full-rate matmuls

LAST_EXEC_NS = None
_CACHE = {}


# ---------------------------------------------------------------- host prep

def _partition_graph(src):
    """Assign nodes to (core, slot) with balanced per-bin edge loads.

    Returns node_core[N], node_slot[N] (slot in [0, 6272)), and the uniform
    tile schedule K_rank[49] (edge tiles per bin rank, same for all cores).
    Bin rank 48 is the dummy bin: real slots [6144:6250), dummies [6250:6272).
    """
    import heapq

    deg = np.bincount(src, minlength=N_NODES)
    order = np.argsort(-deg, kind="stable")  # high degree first

    # snake round-robin over cores -> balanced degree sums, exactly 6250 each
    node_core = np.empty(N_NODES, np.int32)
    blocks = order.reshape(-1, NCORES)  # 6250 rows
    fwd = np.arange(NCORES)
    for i in range(blocks.shape[0]):
        cores = fwd if (i % 2 == 0) else fwd[::-1]
        node_core[blocks[i]] = cores

    node_slot = np.empty(N_NODES, np.int32)
    loads_all = np.empty((NCORES, BINS), np.int64)
    for c in range(NCORES):
        nodes_c = order[node_core[order] == c]  # still degree-sorted desc
        # LPT: assign each node to the least-loaded bin with free slots
        slots_free = [P] * BINS
        slots_free[BINS - 1] = P - DUMMY_PER_CORE  # dummy bin: 106 real slots
        heap = [(0, b) for b in range(BINS)]
        heapq.heapify(heap)
        bin_of = np.empty(len(nodes_c), np.int32)
        slot_in = np.empty(len(nodes_c), np.int32)
        loads = [0] * BINS
        used = [0] * BINS
        degs = deg[nodes_c]
        for i in range(len(nodes_c)):
            while True:
                load, b = heapq.heappop(heap)
                if used[b] < slots_free[b]:
                    break
            bin_of[i] = b
            slot_in[i] = used[b]
            used[b] += 1
            loads[b] += int(degs[i])
            if used[b] < slots_free[b]:
                heapq.heappush(heap, (loads[b], b))
        # rank bins by load desc so the tile-count profile aligns across
        # cores; dummy bin pinned to rank 48
        rest = sorted(range(BINS - 1), key=lambda b: -loads[b])
        rank_of = np.empty(BINS, np.int32)
        for r, b in enumerate(rest):
            rank_of[b] = r
        rank_of[BINS - 1] = BINS - 1
        node_slot[nodes_c] = rank_of[bin_of] * P + slot_in
        for b in range(BINS):
            loads_all[c, rank_of[b]] = loads[b]

    k_rank = np.maximum(1, np.ceil(loads_all.max(axis=0) / P).astype(np.int64))
    return node_core, node_slot, k_rank


def _prepare(x, edge_index, edge_attr):
    src = np.asarray(edge_index[0])
    node_core, node_slot, k_rank = _partition_graph(src)
    tot_tiles = int(k_rank.sum())
    rows = tot_tiles * P  # padded edge rows per core
    tile_off = np.concatenate([[0], np.cumsum(k_rank)])  # per-rank tile offset

    # per-edge destination
    e_core = node_core[src]
    e_slot = node_slot[src]
    e_rank = e_slot // P
    e_srcrel = (e_slot % P).astype(np.float32)
    key = e_core.astype(np.int64) * BINS + e_rank
    order = np.argsort(key, kind="stable")
    key_s = key[order]
    # position within each (core, rank) group
    counts = np.bincount(key_s, minlength=NCORES * BINS)
    starts = np.concatenate([[0], np.cumsum(counts)])[:-1]
    pos = np.arange(N_EDGES, dtype=np.int64) - starts[key_s]
    rank_s = (key_s % BINS).astype(np.int64)
    kg_s = np.asarray(k_rank)[rank_s]
    t_s = pos // P
    e_s = pos % P
    # bin block is [e][t][f]-major: row = bin_row_base + e*K_g + t
    dest_row = tile_off[rank_s] * P + e_s * kg_s + t_s
    dest_core = (key_s // BINS).astype(np.int64)

    edge_dt = ml_dtypes.bfloat16 if EDGE_BF16 else np.float32
    edges_all = np.zeros((NCORES, rows, D), edge_dt)
    edges_all[dest_core, dest_row] = np.asarray(edge_attr)[order].astype(edge_dt)
    srcrel_all = np.zeros((NCORES, rows), np.float32)
    srcrel_dest_row = tile_off[rank_s] * P + t_s * P + e_s
    srcrel_all[dest_core, srcrel_dest_row] = e_srcrel[order]
    # srcrel device layout: [128(e), tot_tiles]
    srcrel_dev = np.ascontiguousarray(
        srcrel_all.reshape(NCORES, tot_tiles, P).transpose(0, 2, 1)
    ).astype(ml_dtypes.bfloat16)

    xt_all = np.zeros((NCORES, D, NODES_PER_CORE), ml_dtypes.bfloat16)
    xt_all[node_core, :, node_slot] = np.asarray(x).astype(ml_dtypes.bfloat16)

    return {
        "node_core": node_core,
        "node_slot": node_slot,
        "k_rank": [int(k) for k in k_rank],
        "tot_tiles": tot_tiles,
        "edges": edges_all,
        "srcrel": srcrel_dev,
        "xt": xt_all,
    }


# ------------------------------------------------------------- device build

def _build(k_rank):
    tot_tiles = int(sum(k_rank))
    nc = bacc.Bacc("TRN2", debug=False, num_devices=NCORES)

    edges_d = nc.declare_dram_parameter("edges", [tot_tiles * P, D], BF16, isOutput=False)
    srcrel_d = nc.declare_dram_parameter("srcrel", [P, tot_tiles], BF16, isOutput=False)
    xt_d = nc.declare_dram_parameter("xt", [D, NODES_PER_CORE], BF16, isOutput=False)
    w1a_d = nc.declare_dram_parameter("w1a", [D, D], BF16, isOutput=False)
    w1b_d = nc.declare_dram_parameter("w1b", [D, D], BF16, isOutput=False)
    w2_d = nc.declare_dram_parameter("w2", [D, D], BF16, isOutput=False)
    w3_d = nc.declare_dram_parameter("w3", [D, D], BF16, isOutput=False)
    b1_d = nc.declare_dram_parameter("b1", [D], F32, isOutput=False)
    b2_d = nc.declare_dram_parameter("b2", [D], F32, isOutput=False)
    b3_d = nc.declare_dram_parameter("b3", [D], F32, isOutput=False)
    g1_d = nc.declare_dram_parameter("g1", [D], F32, isOutput=False)
    g2_d = nc.declare_dram_parameter("g2", [D], F32, isOutput=False)
    be1_d = nc.declare_dram_parameter("be1", [D], F32, isOutput=False)
    be2_d = nc.declare_dram_parameter("be2", [D], F32, isOutput=False)
    out_d = nc.declare_dram_parameter("out", [D, NODES_PER_CORE], F32, isOutput=True)
    dbg_d = nc.declare_dram_parameter("dbg", [NCORES], F32, isOutput=True)

    # collective bounce buffers
    warm_in = nc.dram_tensor("warm_in", [1], F32)
    warm_out = nc.dram_tensor("warm_out", [NCORES], F32, addr_space="Shared")
    cc_in = [nc.dram_tensor(f"cc_in{i}", [2 * D], F32) for i in range(2)]
    cc_out = [
        nc.dram_tensor(f"cc_out{i}", [NCORES * 2 * D], F32, addr_space="Shared")
        for i in range(2)
    ]
    rg = [list(range(NCORES))]

    edt = BF16 if EDGE_BF16 else F32
    inv_n = 1.0 / float(N_NODES)

    with (
        TileContext(nc) as tc,
        tc.tile_pool(name="const", bufs=1) as cpool,
        tc.tile_pool(name="big", bufs=1) as big,
        tc.tile_pool(name="ebin", bufs=6) as ebin_pool,
        tc.tile_pool(name="work", bufs=4) as work,
        tc.tile_pool(name="psum", bufs=4, space="PSUM") as pp,
        tc.tile_pool(name="psum_mlp", bufs=2, space="PSUM") as pmlp,
    ):
        # ---- constants
        iota_i = cpool.tile([P, P], mybir.dt.int32)
        nc.gpsimd.iota(iota_i[:], pattern=[[1, P]], channel_multiplier=0)
        iota_c = cpool.tile([P, P], edt)
        nc.vector.tensor_copy(iota_c[:], iota_i[:])

        srcrel_sb = cpool.tile([P, tot_tiles], BF16)
        nc.sync.dma_start(out=srcrel_sb[:], in_=srcrel_d[:])

        def load_col(dram, nm):
            t = cpool.tile([P, 1], F32, name=f"col_{nm}")
            nc.sync.dma_start(out=t[:], in_=dram[:, None])
            return t

        b1_sb, b2_sb, b3_sb = load_col(b1_d, "b1"), load_col(b2_d, "b2"), load_col(b3_d, "b3")
        g1_sb, g2_sb = load_col(g1_d, "g1"), load_col(g2_d, "g2")
        be1_sb, be2_sb = load_col(be1_d, "be1"), load_col(be2_d, "be2")

        def load_w(dram, nm):
            t = cpool.tile([P, D], BF16, name=f"w_{nm}")
            nc.sync.dma_start(out=t[:], in_=dram[:])
            return t

        w1a_sb = load_w(w1a_d, "w1a")
        w1b_sb = load_w(w1b_d, "w1b")
        w2_sb = load_w(w2_d, "w2")
        w3_sb = load_w(w3_d, "w3")

        # ---- early dummy AllGather to absorb first-collective sync cost
        zz = cpool.tile([1, 1], F32)
        nc.vector.memset(zz[:], 0.0)
        nc.sync.dma_start(out=warm_in[:], in_=zz[0, :])
        nc.gpsimd.collective_compute(
            "AllGather", mybir.AluOpType.bypass, replica_groups=rg,
            ins=[warm_in[:]], outs=[warm_out[:]],
        )
        # ---- big activation buffers (feature-major)
        xt_bf = big.tile([P, NODES_PER_CORE], BF16)
        nc.sync.dma_start(out=xt_bf[:], in_=xt_d[:])
        agg_bf = big.tile([P, NODES_PER_CORE], BF16)
        h1_bf = big.tile([P, NODES_PER_CORE], BF16)
        h2_bf = big.tile([P, NODES_PER_CORE], BF16)

        # per-layer stat partials: [128, tiles] (sum, sumsq)
        s_part = [cpool.tile([P, MLP_TILES], F32, name=f"s_part{i}") for i in range(2)]
        q_part = [cpool.tile([P, MLP_TILES], F32, name=f"q_part{i}") for i in range(2)]

        def mlp_tile_cols(j):
            c0 = j * MLP_TILE
            c1 = min(NODES_PER_CORE, c0 + MLP_TILE)
            return c0, c1

        def silu_layer(li, j, ps, h_out, b_sb):
            """SiLU(ps + b) -> h_out cols of tile j, plus sum/sumsq partials
            (dummy slots excluded on the last tile)."""
            c0, c1 = mlp_tile_cols(j)
            w = c1 - c0
            real_w = w if j < MLP_TILES - 1 else (w - DUMMY_PER_CORE)
            nc.scalar.activation(
                out=h_out[:, c0 : c0 + real_w], in_=ps[:, :real_w],
                func=mybir.ActivationFunctionType.Silu,
                bias=b_sb[:, :1], scale=1.0,
                accum_out=s_part[li][:, j : j + 1],
            )
            if real_w < w:
                nc.scalar.activation(
                    out=h_out[:, c0 + real_w : c1], in_=ps[:, real_w:w],
                    func=mybir.ActivationFunctionType.Silu,
                    bias=b_sb[:, :1], scale=1.0,
                )
            sq = work.tile([P, MLP_TILE], BF16, tag="sq")
            nc.scalar.activation(
                out=sq[:, :real_w], in_=h_out[:, c0 : c0 + real_w],
                func=mybir.ActivationFunctionType.Square,
                accum_out=q_part[li][:, j : j + 1],
            )

        # ---- edge phase with interleaved layer-1 MLP
        tile_idx = 0
        for g in range(BINS):
            kg = k_rank[g]
            ebin_c = ebin_pool.tile([P, kg, D], BF16, tag="ebin")
            nc.sync.dma_start(
                out=ebin_c.rearrange("e t f -> e (t f)"),
                in_=edges_d[tile_idx * P : (tile_idx + kg) * P, :].rearrange(
                    "(e q) f -> e (q f)", e=P
                ),
            )

            ps_agg = pp.tile([P, P], F32, tag="ps_agg")
            t = 0
            while t < kg:
                ch = min(OH_CHUNK, kg - t)
                oh = work.tile([P, OH_CHUNK, P], edt, tag="oh")
                nc.vector.tensor_tensor(
                    out=oh[:, :ch, :],
                    in0=iota_c[:, None, :].to_broadcast([P, ch, P]),
                    in1=srcrel_sb[:, tile_idx : tile_idx + ch, None].to_broadcast(
                        [P, ch, P]
                    ),
                    op=mybir.AluOpType.is_equal,
                )
                for u in range(ch):
                    nc.tensor.matmul(
                        out=ps_agg[:], lhsT=ebin_c[:, t + u, :], rhs=oh[:, u, :],
                        start=(t + u == 0), stop=(t + u == kg - 1),
                    )
                t += ch
                tile_idx += ch
            nc.scalar.copy(agg_bf[:, g * P : (g + 1) * P], ps_agg[:])

            if g % 4 == 3 or g == BINS - 1:
                j = g // 4
                c0, c1 = mlp_tile_cols(j)
                ps1 = pmlp.tile([P, MLP_TILE], F32, tag="ps_mlp")
                nc.tensor.matmul(
                    out=ps1[:, : c1 - c0], lhsT=w1a_sb[:], rhs=xt_bf[:, c0:c1],
                    start=True, stop=False,
                )
                nc.tensor.matmul(
                    out=ps1[:, : c1 - c0], lhsT=w1b_sb[:], rhs=agg_bf[:, c0:c1],
                    start=False, stop=True,
                )
                silu_layer(0, j, ps1, h1_bf, b1_sb)

        # ---- BN stats + weight folding helper
        def bn_fold(li, w_sb, g_sb, be_sb, b_next_sb):
            """AllGather (sum, sumsq), compute scale/shift, fold into w_sb
            (rows) producing (w_fold, b_fold)."""
            s_loc = cpool.tile([P, 1], F32, tag=f"s_loc{li}")
            q_loc = cpool.tile([P, 1], F32, tag=f"q_loc{li}")
            nc.vector.reduce_sum(out=s_loc[:], in_=s_part[li][:], axis=mybir.AxisListType.X)
            nc.vector.reduce_sum(out=q_loc[:], in_=q_part[li][:], axis=mybir.AxisListType.X)
            nc.sync.dma_start(out=cc_in[li][0:D], in_=s_loc[:, 0])
            nc.sync.dma_start(out=cc_in[li][D : 2 * D], in_=q_loc[:, 0])
            nc.gpsimd.collective_compute(
                "AllGather", mybir.AluOpType.bypass, replica_groups=rg,
                ins=[cc_in[li][:]], outs=[cc_out[li][:]],
            )
            gath = work.tile([P, 2, NCORES], F32, tag=f"gath{li}")
            cc_view = cc_out[li].rearrange("(r kf) -> kf r", kf=2 * D)
            nc.sync.dma_start(out=gath[:, 0, :], in_=cc_view[0:D, :])
            nc.sync.dma_start(out=gath[:, 1, :], in_=cc_view[D : 2 * D, :])
            mean = cpool.tile([P, 1], F32, tag=f"mean{li}")
            msq = cpool.tile([P, 1], F32, tag=f"msq{li}")
            nc.vector.reduce_sum(out=mean[:], in_=gath[:, 0, :], axis=mybir.AxisListType.X)
            nc.vector.tensor_scalar_mul(mean[:], mean[:], inv_n)
            nc.vector.reduce_sum(out=msq[:], in_=gath[:, 1, :], axis=mybir.AxisListType.X)
            nc.vector.tensor_scalar_mul(msq[:], msq[:], inv_n)
            var = cpool.tile([P, 1], F32, tag=f"var{li}")
            nc.vector.tensor_tensor(
                out=var[:], in0=mean[:], in1=mean[:], op=mybir.AluOpType.mult
            )
            nc.vector.tensor_tensor(
                out=var[:], in0=msq[:], in1=var[:], op=mybir.AluOpType.subtract
            )
            nc.vector.tensor_scalar_add(var[:], var[:], BN_EPS)
            std = cpool.tile([P, 1], F32, tag=f"std{li}")
            nc.scalar.sqrt(std[:], var[:])
            rstd = cpool.tile([P, 1], F32, tag=f"rstd{li}")
            nc.vector.reciprocal(rstd[:], std[:])
            scale = cpool.tile([P, 1], F32, tag=f"scale{li}")
            nc.vector.tensor_tensor(
                out=scale[:], in0=g_sb[:], in1=rstd[:], op=mybir.AluOpType.mult
            )
            shift = cpool.tile([P, 1], F32, tag=f"shift{li}")
            nc.vector.tensor_tensor(
                out=shift[:], in0=mean[:], in1=scale[:], op=mybir.AluOpType.mult
            )
            nc.vector.tensor_tensor(
                out=shift[:], in0=be_sb[:], in1=shift[:], op=mybir.AluOpType.subtract
            )
            w_fold = cpool.tile([P, D], BF16, tag=f"wf{li}")
            nc.vector.tensor_scalar(
                out=w_fold[:], in0=w_sb[:], scalar1=scale[:, :1], scalar2=None,
                op0=mybir.AluOpType.mult,
            )
            shift_bf = cpool.tile([P, 1], BF16, tag=f"shift_bf{li}")
            nc.vector.tensor_copy(shift_bf[:], shift[:])
            ps_b = pmlp.tile([P, MLP_TILE], F32, tag="ps_mlp")
            nc.tensor.matmul(out=ps_b[:, :1], lhsT=w_sb[:], rhs=shift_bf[:], start=True, stop=True)
            b_fold = cpool.tile([P, 1], F32, tag=f"bf{li}")
            nc.vector.tensor_tensor(
                out=b_fold[:], in0=ps_b[:, :1], in1=b_next_sb[:], op=mybir.AluOpType.add
            )
            return w_fold, b_fold

        # ---- layer 2
        w2f, b2f = bn_fold(0, w2_sb, g1_sb, be1_sb, b2_sb)
        for j in range(MLP_TILES):
            c0, c1 = mlp_tile_cols(j)
            ps2 = pmlp.tile([P, MLP_TILE], F32, tag="ps_mlp")
            nc.tensor.matmul(
                out=ps2[:, : c1 - c0], lhsT=w2f[:], rhs=h1_bf[:, c0:c1],
                start=True, stop=True,
            )
            silu_layer(1, j, ps2, h2_bf, b2f)

        # ---- layer 3
        w3f, b3f = bn_fold(1, w3_sb, g2_sb, be2_sb, b3_sb)
        for j in range(MLP_TILES):
            c0, c1 = mlp_tile_cols(j)
            ps3 = pmlp.tile([P, MLP_TILE], F32, tag="ps_mlp")
            nc.tensor.matmul(
                out=ps3[:, : c1 - c0], lhsT=w3f[:], rhs=h2_bf[:, c0:c1],
                start=True, stop=True,
            )
            o = work.tile([P, MLP_TILE], F32, tag="otile")
            nc.scalar.activation(
                out=o[:, : c1 - c0], in_=ps3[:, : c1 - c0],
                func=mybir.ActivationFunctionType.Identity,
                bias=b3f[:, :1], scale=1.0,
            )
            nc.sync.dma_start(out=out_d[:, c0:c1], in_=o[:, : c1 - c0])

        nc.sync.dma_start(out=dbg_d[:], in_=warm_out[:])

    nc.finalize()
    return nc


# ------------------------------------------------------------------- driver

def kernel(x, edge_index, edge_attr, W1, b1, g1, be1, W2, b2, g2, be2, W3, b3):
    global LAST_EXEC_NS
    x = np.asarray(x)
    prep = _prepare(x, np.asarray(edge_index), np.asarray(edge_attr))
    key = tuple(prep["k_rank"])
    if key not in _CACHE:
        _CACHE[key] = _build(prep["k_rank"])
    nc = _CACHE[key]

    W1 = np.asarray(W1, np.float32)
    bf = ml_dtypes.bfloat16
    shared = {
        "w1a": np.ascontiguousarray(W1[:D]).astype(bf),
        "w1b": np.ascontiguousarray(W1[D:]).astype(bf),
        "w2": np.asarray(W2, np.float32).astype(bf),
        "w3": np.asarray(W3, np.float32).astype(bf),
        "b1": np.asarray(b1, np.float32), "b2": np.asarray(b2, np.float32),
        "b3": np.asarray(b3, np.float32),
        "g1": np.asarray(g1, np.float32), "g2": np.asarray(g2, np.float32),
        "be1": np.asarray(be1, np.float32), "be2": np.asarray(be2, np.float32),
    }
    in_maps = []
    for c in range(NCORES):
        m = dict(shared)
        m["edges"] = prep["edges"][c]
        m["srcrel"] = prep["srcrel"][c]
        m["xt"] = prep["xt"][c]
        in_maps.append(m)

    trace = bool(os.environ.get("KERNEL_TRACE"))
    res = run_bass_kernel_spmd(
        nc, in_maps, core_ids=list(range(NCORES)), trace=trace
    )
    LAST_EXEC_NS = res.exec_time_ns

    outs = np.stack([np.asarray(res.results[c]["out"]) for c in range(NCORES)])
    # [core, D, slot] -> [N, D]
    out = outs[prep["node_core"], :, prep["node_slot"]]
    return np.ascontiguousarray(out.astype(np.float32))


# revision 11
# speedup vs baseline: 1.5035x; 1.0050x over previous
"""GNN message-passing (NodeModel) Trainium2 kernel, 8-core SPMD.

reference:
    agg = segment_sum(edge_attr, src, N)            # [N, D] scatter-add
    h = concat([x, agg], 1)                          # [N, 2D]
    h = BN(SiLU(h @ W1 + b1)); h = BN(SiLU(h @ W2 + b2)); out = h @ W3 + b3
    (BN in training mode: batch stats over all N nodes)

Strategy:
  - Host: permute nodes into 8 cores x 49 bins x 128 slots with per-bin edge
    loads balanced (LPT bin packing), bucket edges by owner bin, pad each bin
    to a whole number of 128-edge tiles with a cross-core-uniform tile
    schedule. All activations live feature-major ([D, nodes]) on device.
  - Device per core: for each bin, stream its edge tiles; one-hot(srcrel)
    matmuls accumulate the segment-sum in PSUM ([D, 128nodes]); MLP runs
    feature-major so BN stats are per-partition row sums (activation
    accum_out), BN scale/shift are folded into the next layer's weights.
    Global BN stats via two tiny AllGathers (plus an early warmup AllGather
    that absorbs the first-collective sync cost under the edge stream).
  - Dummy node slots (176 pad) are excluded from BN stats exactly.
"""

import os
import sys

for _p in ("/opt/trn_rl_repo",):
    if _p not in sys.path:
        sys.path.append(_p)

import numpy as np
import ml_dtypes

import concourse.bacc as bacc
from concourse import mybir
from concourse.tile import TileContext
from concourse.bass_utils import run_bass_kernel_spmd

# problem constants (hardcoded per contract)
N_NODES = 50000
N_EDGES = 600000
D = 128
BN_EPS = 1e-5
P = 128
NCORES = 8
BINS = 49                      # bins (128 nodes each) per core
NODES_PER_CORE = BINS * P      # 6272
REAL_PER_CORE = N_NODES // NCORES   # 6250
DUMMY_PER_CORE = NODES_PER_CORE - REAL_PER_CORE  # 22
MLP_TILE = 512
MLP_TILES = 13                 # 12 x 512 + 1 x 128
OH_CHUNK = 6                   # onehot tiles generated per DVE op

F32 = mybir.dt.float32
BF16 = mybir.dt.bfloat16

EDGE_BF16 = True   # convert edge tiles to bf16 on device for full-rate matmuls

LAST_EXEC_NS = None
_CACHE = {}


# ---------------------------------------------------------------- host prep

def _partition_graph(src):
    """Assign nodes to (core, slot) using a shared per-slot capacity profile.

    Nodes are sorted by degree (desc) and dealt round-robin across all 392
    bins slot-by-slot, so every bin's slot s holds nodes of nearly equal
    degree; cap[s] = max degree at slot s. With a capacity profile shared by
    ALL bins, the edge-row -> node-slot map is data-independent, so the
    one-hot matmul operands are compile-time constants.

    Returns node_core[N], node_slot[N], cap[128] (per-slot edge capacity),
    and K (edge tiles per bin). Dummy slots: each core's last bin,
    slots [106:128).
    """
    deg = np.bincount(src, minlength=N_NODES)
    order = np.argsort(-deg, kind="stable")  # high degree first

    TOT_BINS = NCORES * BINS
    # position list: (bin, slot) in slot-major order, dummy positions last
    # bins are numbered c*BINS+b; core c's last bin is c*BINS+BINS-1
    last_bins = np.array([c * BINS + BINS - 1 for c in range(NCORES)])
    is_last = np.zeros(TOT_BINS, bool)
    is_last[last_bins] = True
    positions = []
    for s_ in range(P):
        for b in range(TOT_BINS):
            if s_ >= P - DUMMY_PER_CORE and is_last[b]:
                continue
            positions.append((b, s_))
    assert len(positions) == N_NODES
    pos_bin = np.array([p[0] for p in positions], np.int32)
    pos_slot = np.array([p[1] for p in positions], np.int32)

    node_core = np.empty(N_NODES, np.int32)
    node_slot = np.empty(N_NODES, np.int32)
    node_core[order] = pos_bin // BINS
    node_slot[order] = (pos_bin % BINS) * P + pos_slot

    cap = np.zeros(P, np.int64)
    degs_sorted = deg[order]
    for i in range(N_NODES):
        s_ = pos_slot[i]
        if degs_sorted[i] > cap[s_]:
            cap[s_] = degs_sorted[i]
    total = int(cap.sum())
    K = max(1, (total + P - 1) // P)
    return node_core, node_slot, cap, K


def _prepare(x, edge_index, edge_attr):
    src = np.asarray(edge_index[0])
    node_core, node_slot, cap, K = _partition_graph(src)
    rows_per_bin = K * P
    rows = BINS * rows_per_bin
    slot_base = np.concatenate([[0], np.cumsum(cap)])  # [129]

    # linear edge slot within bin: l = slot_base[s] + pos_within_node
    # dram row within bin block (e-major layout): (l % P) * K + (l // P)
    order = np.argsort(src, kind="stable")
    src_s = src[order]
    counts = np.bincount(src_s, minlength=N_NODES)
    starts = np.concatenate([[0], np.cumsum(counts)])[:-1]
    pos = np.arange(N_EDGES, dtype=np.int64) - starts[src_s]

    e_core = node_core[src_s].astype(np.int64)
    e_slotg = node_slot[src_s].astype(np.int64)
    e_bin = e_slotg // P
    e_s = e_slotg % P
    l = slot_base[e_s] + pos
    dest_row = e_bin * rows_per_bin + (l % P) * K + (l // P)

    edge_dt = ml_dtypes.bfloat16 if EDGE_BF16 else np.float32
    edges_all = np.zeros((NCORES, rows, D), edge_dt)
    edges_all[e_core, dest_row] = np.asarray(edge_attr)[order].astype(edge_dt)

    # constant one-hot operands: M[e, t, n] = (slot_of(t*P + e) == n)
    slot_of = np.searchsorted(slot_base, np.arange(rows_per_bin), side="right") - 1
    slot_of = np.minimum(slot_of, P - 1)
    M = np.zeros((P, K, P), edge_dt)
    lin = np.arange(rows_per_bin)
    M[lin % P, lin // P, slot_of] = 1.0
    # rows past the real capacity: zero attr anyway; keep mapping to slot 127

    xt_all = np.zeros((NCORES, D, NODES_PER_CORE), ml_dtypes.bfloat16)
    xt_all[node_core, :, node_slot] = np.asarray(x).astype(ml_dtypes.bfloat16)

    return {
        "node_core": node_core,
        "node_slot": node_slot,
        "K": K,
        "edges": edges_all,
        "onehot": np.ascontiguousarray(M.reshape(P, K * P)),
        "xt": xt_all,
    }


# ------------------------------------------------------------- device build

def _build(K):
    tot_tiles = BINS * K
    nc = bacc.Bacc("TRN2", debug=False, num_devices=NCORES)

    edges_d = nc.declare_dram_parameter("edges", [tot_tiles * P, D], BF16, isOutput=False)
    onehot_d = nc.declare_dram_parameter("onehot", [P, K * P], BF16, isOutput=False)
    xt_d = nc.declare_dram_parameter("xt", [D, NODES_PER_CORE], BF16, isOutput=False)
    w1a_d = nc.declare_dram_parameter("w1a", [D, D], BF16, isOutput=False)
    w1b_d = nc.declare_dram_parameter("w1b", [D, D], BF16, isOutput=False)
    w2_d = nc.declare_dram_parameter("w2", [D, D], BF16, isOutput=False)
    w3_d = nc.declare_dram_parameter("w3", [D, D], BF16, isOutput=False)
    b1_d = nc.declare_dram_parameter("b1", [D], F32, isOutput=False)
    b2_d = nc.declare_dram_parameter("b2", [D], F32, isOutput=False)
    b3_d = nc.declare_dram_parameter("b3", [D], F32, isOutput=False)
    g1_d = nc.declare_dram_parameter("g1", [D], F32, isOutput=False)
    g2_d = nc.declare_dram_parameter("g2", [D], F32, isOutput=False)
    be1_d = nc.declare_dram_parameter("be1", [D], F32, isOutput=False)
    be2_d = nc.declare_dram_parameter("be2", [D], F32, isOutput=False)
    out_d = nc.declare_dram_parameter("out", [D, NODES_PER_CORE], F32, isOutput=True)
    dbg_d = nc.declare_dram_parameter("dbg", [NCORES], F32, isOutput=True)

    # collective bounce buffers
    warm_in = nc.dram_tensor("warm_in", [1], F32)
    warm_out = nc.dram_tensor("warm_out", [NCORES], F32, addr_space="Shared")
    cc_in = [nc.dram_tensor(f"cc_in{i}", [2 * D], F32) for i in range(2)]
    cc_out = [
        nc.dram_tensor(f"cc_out{i}", [NCORES * 2 * D], F32, addr_space="Shared")
        for i in range(2)
    ]
    rg = [list(range(NCORES))]

    edt = BF16 if EDGE_BF16 else F32
    inv_n = 1.0 / float(N_NODES)

    with (
        TileContext(nc) as tc,
        tc.tile_pool(name="const", bufs=1) as cpool,
        tc.tile_pool(name="big", bufs=1) as big,
        tc.tile_pool(name="ebin", bufs=6) as ebin_pool,
        tc.tile_pool(name="work", bufs=4) as work,
        tc.tile_pool(name="psum", bufs=4, space="PSUM") as pp,
        tc.tile_pool(name="psum_mlp", bufs=2, space="PSUM") as pmlp,
    ):
        # ---- constants
        oh_sb = cpool.tile([P, K, P], BF16)
        nc.sync.dma_start(out=oh_sb.rearrange("e t f -> e (t f)"), in_=onehot_d[:])

        def load_col(dram, nm):
            t = cpool.tile([P, 1], F32, name=f"col_{nm}")
            nc.sync.dma_start(out=t[:], in_=dram[:, None])
            return t

        b1_sb, b2_sb, b3_sb = load_col(b1_d, "b1"), load_col(b2_d, "b2"), load_col(b3_d, "b3")
        g1_sb, g2_sb = load_col(g1_d, "g1"), load_col(g2_d, "g2")
        be1_sb, be2_sb = load_col(be1_d, "be1"), load_col(be2_d, "be2")

        def load_w(dram, nm):
            t = cpool.tile([P, D], BF16, name=f"w_{nm}")
            nc.sync.dma_start(out=t[:], in_=dram[:])
            return t

        w1a_sb = load_w(w1a_d, "w1a")
        w1b_sb = load_w(w1b_d, "w1b")
        w2_sb = load_w(w2_d, "w2")
        w3_sb = load_w(w3_d, "w3")

        # ---- early dummy AllGather to absorb first-collective sync cost
        zz = cpool.tile([1, 1], F32)
        nc.vector.memset(zz[:], 0.0)
        nc.sync.dma_start(out=warm_in[:], in_=zz[0, :])
        nc.gpsimd.collective_compute(
            "AllGather", mybir.AluOpType.bypass, replica_groups=rg,
            ins=[warm_in[:]], outs=[warm_out[:]],
        )
        # ---- big activation buffers (feature-major)
        xt_bf = big.tile([P, NODES_PER_CORE], BF16)
        nc.sync.dma_start(out=xt_bf[:], in_=xt_d[:])
        agg_bf = big.tile([P, NODES_PER_CORE], BF16)
        h1_bf = big.tile([P, NODES_PER_CORE], BF16)
        h2_bf = big.tile([P, NODES_PER_CORE], BF16)

        # per-layer stat partials: [128, tiles] (sum, sumsq)
        s_part = [cpool.tile([P, MLP_TILES], F32, name=f"s_part{i}") for i in range(2)]
        q_part = [cpool.tile([P, MLP_TILES], F32, name=f"q_part{i}") for i in range(2)]

        def mlp_tile_cols(j):
            c0 = j * MLP_TILE
            c1 = min(NODES_PER_CORE, c0 + MLP_TILE)
            return c0, c1

        def silu_layer(li, j, ps, h_out, b_sb):
            """SiLU(ps + b) -> h_out cols of tile j, plus sum/sumsq partials
            (dummy slots excluded on the last tile)."""
            c0, c1 = mlp_tile_cols(j)
            w = c1 - c0
            real_w = w if j < MLP_TILES - 1 else (w - DUMMY_PER_CORE)
            nc.scalar.activation(
                out=h_out[:, c0 : c0 + real_w], in_=ps[:, :real_w],
                func=mybir.ActivationFunctionType.Silu,
                bias=b_sb[:, :1], scale=1.0,
                accum_out=s_part[li][:, j : j + 1],
            )
            if real_w < w:
                nc.scalar.activation(
                    out=h_out[:, c0 + real_w : c1], in_=ps[:, real_w:w],
                    func=mybir.ActivationFunctionType.Silu,
                    bias=b_sb[:, :1], scale=1.0,
                )
            sq = work.tile([P, MLP_TILE], BF16, tag="sq")
            nc.scalar.activation(
                out=sq[:, :real_w], in_=h_out[:, c0 : c0 + real_w],
                func=mybir.ActivationFunctionType.Square,
                accum_out=q_part[li][:, j : j + 1],
            )

        # ---- edge phase with interleaved layer-1 MLP
        tile_idx = 0
        for g in range(BINS):
            kg = K
            ebin_c = ebin_pool.tile([P, kg, D], BF16, tag="ebin")
            nc.sync.dma_start(
                out=ebin_c.rearrange("e t f -> e (t f)"),
                in_=edges_d[tile_idx * P : (tile_idx + kg) * P, :].rearrange(
                    "(e q) f -> e (q f)", e=P
                ),
            )

            ps_agg = pp.tile([P, P], F32, tag="ps_agg")
            for t in range(kg):
                nc.tensor.matmul(
                    out=ps_agg[:], lhsT=ebin_c[:, t, :], rhs=oh_sb[:, t, :],
                    start=(t == 0), stop=(t == kg - 1),
                )
            tile_idx += kg
            nc.scalar.copy(agg_bf[:, g * P : (g + 1) * P], ps_agg[:])

            if g % 4 == 3 or g == BINS - 1:
                j = g // 4
                c0, c1 = mlp_tile_cols(j)
                ps1 = pmlp.tile([P, MLP_TILE], F32, tag="ps_mlp")
                nc.tensor.matmul(
                    out=ps1[:, : c1 - c0], lhsT=w1a_sb[:], rhs=xt_bf[:, c0:c1],
                    start=True, stop=False,
                )
                nc.tensor.matmul(
                    out=ps1[:, : c1 - c0], lhsT=w1b_sb[:], rhs=agg_bf[:, c0:c1],
                    start=False, stop=True,
                )
                silu_layer(0, j, ps1, h1_bf, b1_sb)

        # ---- BN stats + weight folding helper
        def bn_fold(li, w_sb, g_sb, be_sb, b_next_sb):
            """AllGather (sum, sumsq), compute scale/shift, fold into w_sb
            (rows) producing (w_fold, b_fold)."""
            s_loc = cpool.tile([P, 1], F32, tag=f"s_loc{li}")
            q_loc = cpool.tile([P, 1], F32, tag=f"q_loc{li}")
            nc.vector.reduce_sum(out=s_loc[:], in_=s_part[li][:], axis=mybir.AxisListType.X)
            nc.vector.reduce_sum(out=q_loc[:], in_=q_part[li][:], axis=mybir.AxisListType.X)
            nc.sync.dma_start(out=cc_in[li][0:D], in_=s_loc[:, 0])
            nc.sync.dma_start(out=cc_in[li][D : 2 * D], in_=q_loc[:, 0])
            nc.gpsimd.collective_compute(
                "AllGather", mybir.AluOpType.bypass, replica_groups=rg,
                ins=[cc_in[li][:]], outs=[cc_out[li][:]],
            )
            gath = work.tile([P, 2, NCORES], F32, tag=f"gath{li}")
            cc_view = cc_out[li].rearrange("(r kf) -> kf r", kf=2 * D)
            nc.sync.dma_start(out=gath[:, 0, :], in_=cc_view[0:D, :])
            nc.sync.dma_start(out=gath[:, 1, :], in_=cc_view[D : 2 * D, :])
            mean = cpool.tile([P, 1], F32, tag=f"mean{li}")
            msq = cpool.tile([P, 1], F32, tag=f"msq{li}")
            nc.vector.reduce_sum(out=mean[:], in_=gath[:, 0, :], axis=mybir.AxisListType.X)
            nc.vector.tensor_scalar_mul(mean[:], mean[:], inv_n)
            nc.vector.reduce_sum(out=msq[:], in_=gath[:, 1, :], axis=mybir.AxisListType.X)
            nc.vector.tensor_scalar_mul(msq[:], msq[:], inv_n)
            var = cpool.tile([P, 1], F32, tag=f"var{li}")
            nc.vector.tensor_tensor(
                out=var[:], in0=mean[:], in1=mean[:], op=mybir.AluOpType.mult
            )
            nc.vector.tensor_tensor(
                out=var[:], in0=msq[:], in1=var[:], op=mybir.AluOpType.subtract
            )
            nc.vector.tensor_scalar_add(var[:], var[:], BN_EPS)
            std = cpool.tile([P, 1], F32, tag=f"std{li}")
            nc.scalar.sqrt(std[:], var[:])
            rstd = cpool.tile([P, 1], F32, tag=f"rstd{li}")
            nc.vector.reciprocal(rstd[:], std[:])
            scale = cpool.tile([P, 1], F32, tag=f"scale{li}")
            nc.vector.tensor_tensor(
                out=scale[:], in0=g_sb[:], in1=rstd[:], op=mybir.AluOpType.mult
            )
            shift = cpool.tile([P, 1], F32, tag=f"shift{li}")
            nc.vector.tensor_tensor(
                out=shift[:], in0=mean[:], in1=scale[:], op=mybir.AluOpType.mult
            )
            nc.vector.tensor_tensor(
                out=shift[:], in0=be_sb[:], in1=shift[:], op=mybir.AluOpType.subtract
            )
            w_fold = cpool.tile([P, D], BF16, tag=f"wf{li}")
            nc.vector.tensor_scalar(
                out=w_fold[:], in0=w_sb[:], scalar1=scale[:, :1], scalar2=None,
                op0=mybir.AluOpType.mult,
            )
            shift_bf = cpool.tile([P, 1], BF16, tag=f"shift_bf{li}")
            nc.vector.tensor_copy(shift_bf[:], shift[:])
            ps_b = pmlp.tile([P, MLP_TILE], F32, tag="ps_mlp")
            nc.tensor.matmul(out=ps_b[:, :1], lhsT=w_sb[:], rhs=shift_bf[:], start=True, stop=True)
            b_fold = cpool.tile([P, 1], F32, tag=f"bf{li}")
            nc.vector.tensor_tensor(
                out=b_fold[:], in0=ps_b[:, :1], in1=b_next_sb[:], op=mybir.AluOpType.add
            )
            return w_fold, b_fold

        # ---- layer 2
        w2f, b2f = bn_fold(0, w2_sb, g1_sb, be1_sb, b2_sb)
        for j in range(MLP_TILES):
            c0, c1 = mlp_tile_cols(j)
            ps2 = pmlp.tile([P, MLP_TILE], F32, tag="ps_mlp")
            nc.tensor.matmul(
                out=ps2[:, : c1 - c0], lhsT=w2f[:], rhs=h1_bf[:, c0:c1],
                start=True, stop=True,
            )
            silu_layer(1, j, ps2, h2_bf, b2f)

        # ---- layer 3
        w3f, b3f = bn_fold(1, w3_sb, g2_sb, be2_sb, b3_sb)
        for j in range(MLP_TILES):
            c0, c1 = mlp_tile_cols(j)
            ps3 = pmlp.tile([P, MLP_TILE], F32, tag="ps_mlp")
            nc.tensor.matmul(
                out=ps3[:, : c1 - c0], lhsT=w3f[:], rhs=h2_bf[:, c0:c1],
                start=True, stop=True,
            )
            o = work.tile([P, MLP_TILE], F32, tag="otile")
            nc.scalar.activation(
                out=o[:, : c1 - c0], in_=ps3[:, : c1 - c0],
                func=mybir.ActivationFunctionType.Identity,
                bias=b3f[:, :1], scale=1.0,
            )
            nc.sync.dma_start(out=out_d[:, c0:c1], in_=o[:, : c1 - c0])

        nc.sync.dma_start(out=dbg_d[:], in_=warm_out[:])

    nc.finalize()
    return nc


# ------------------------------------------------------------------- driver

def kernel(x, edge_index, edge_attr, W1, b1, g1, be1, W2, b2, g2, be2, W3, b3):
    global LAST_EXEC_NS
    x = np.asarray(x)
    prep = _prepare(x, np.asarray(edge_index), np.asarray(edge_attr))
    key = prep["K"]
    if key not in _CACHE:
        _CACHE[key] = _build(prep["K"])
    nc = _CACHE[key]

    W1 = np.asarray(W1, np.float32)
    bf = ml_dtypes.bfloat16
    shared = {
        "w1a": np.ascontiguousarray(W1[:D]).astype(bf),
        "w1b": np.ascontiguousarray(W1[D:]).astype(bf),
        "w2": np.asarray(W2, np.float32).astype(bf),
        "w3": np.asarray(W3, np.float32).astype(bf),
        "b1": np.asarray(b1, np.float32), "b2": np.asarray(b2, np.float32),
        "b3": np.asarray(b3, np.float32),
        "g1": np.asarray(g1, np.float32), "g2": np.asarray(g2, np.float32),
        "be1": np.asarray(be1, np.float32), "be2": np.asarray(be2, np.float32),
    }
    in_maps = []
    for c in range(NCORES):
        m = dict(shared)
        m["edges"] = prep["edges"][c]
        m["onehot"] = prep["onehot"]
        m["xt"] = prep["xt"][c]
        in_maps.append(m)

    trace = bool(os.environ.get("KERNEL_TRACE"))
    res = run_bass_kernel_spmd(
        nc, in_maps, core_ids=list(range(NCORES)), trace=trace
    )
    LAST_EXEC_NS = res.exec_time_ns

    outs = np.stack([np.asarray(res.results[c]["out"]) for c in range(NCORES)])
    # [core, D, slot] -> [N, D]
    out = outs[prep["node_core"], :, prep["node_slot"]]
    return np.ascontiguousarray(out.astype(np.float32))


# revision 12
# speedup vs baseline: 1.6719x; 1.1120x over previous
"""GNN message-passing (NodeModel) Trainium2 kernel, 8-core SPMD.

reference:
    agg = segment_sum(edge_attr, src, N)            # [N, D] scatter-add
    h = concat([x, agg], 1)                          # [N, 2D]
    h = BN(SiLU(h @ W1 + b1)); h = BN(SiLU(h @ W2 + b2)); out = h @ W3 + b3
    (BN in training mode: batch stats over all N nodes)

Strategy:
  - Host: permute nodes into 8 cores x 49 bins x 128 slots with per-bin edge
    loads balanced (LPT bin packing), bucket edges by owner bin, pad each bin
    to a whole number of 128-edge tiles with a cross-core-uniform tile
    schedule. All activations live feature-major ([D, nodes]) on device.
  - Device per core: for each bin, stream its edge tiles; one-hot(srcrel)
    matmuls accumulate the segment-sum in PSUM ([D, 128nodes]); MLP runs
    feature-major so BN stats are per-partition row sums (activation
    accum_out), BN scale/shift are folded into the next layer's weights.
    Global BN stats via two tiny AllGathers (plus an early warmup AllGather
    that absorbs the first-collective sync cost under the edge stream).
  - Dummy node slots (176 pad) are excluded from BN stats exactly.
"""

import os
import sys

for _p in ("/opt/trn_rl_repo",):
    if _p not in sys.path:
        sys.path.append(_p)

import numpy as np
import ml_dtypes

import concourse.bacc as bacc
from concourse import mybir
from concourse.tile import TileContext
from concourse.bass_utils import run_bass_kernel_spmd

# problem constants (hardcoded per contract)
N_NODES = 50000
N_EDGES = 600000
D = 128
BN_EPS = 1e-5
P = 128
NCORES = 8
BINS = 49                      # bins (128 nodes each) per core
NODES_PER_CORE = BINS * P      # 6272
REAL_PER_CORE = N_NODES // NCORES   # 6250
DUMMY_PER_CORE = NODES_PER_CORE - REAL_PER_CORE  # 22
MLP_TILE = 512
MLP_TILES = 13                 # 12 x 512 + 1 x 128
OH_CHUNK = 6                   # onehot tiles generated per DVE op

F32 = mybir.dt.float32
BF16 = mybir.dt.bfloat16

EDGE_BF16 = True   # convert edge tiles to bf16 on device for full-rate matmuls

LAST_EXEC_NS = None
_CACHE = {}


# ---------------------------------------------------------------- host prep

def _partition_graph(src):
    """Assign nodes to (core, slot) using a shared per-slot capacity profile.

    Nodes are sorted by degree (desc) and dealt round-robin across all 392
    bins slot-by-slot, so every bin's slot s holds nodes of nearly equal
    degree; cap[s] = max degree at slot s. With a capacity profile shared by
    ALL bins, the edge-row -> node-slot map is data-independent, so the
    one-hot matmul operands are compile-time constants.

    Returns node_core[N], node_slot[N], cap[128] (per-slot edge capacity),
    and K (edge tiles per bin). Dummy slots: each core's last bin,
    slots [106:128).
    """
    deg = np.bincount(src, minlength=N_NODES)
    order = np.argsort(-deg, kind="stable")  # high degree first

    TOT_BINS = NCORES * BINS
    # position list: (bin, slot) in slot-major order, dummy positions last
    # bins are numbered c*BINS+b; core c's last bin is c*BINS+BINS-1
    last_bins = np.array([c * BINS + BINS - 1 for c in range(NCORES)])
    is_last = np.zeros(TOT_BINS, bool)
    is_last[last_bins] = True
    positions = []
    for s_ in range(P):
        for b in range(TOT_BINS):
            if s_ >= P - DUMMY_PER_CORE and is_last[b]:
                continue
            positions.append((b, s_))
    assert len(positions) == N_NODES
    pos_bin = np.array([p[0] for p in positions], np.int32)
    pos_slot = np.array([p[1] for p in positions], np.int32)

    node_core = np.empty(N_NODES, np.int32)
    node_slot = np.empty(N_NODES, np.int32)
    node_core[order] = pos_bin // BINS
    node_slot[order] = (pos_bin % BINS) * P + pos_slot

    cap = np.zeros(P, np.int64)
    degs_sorted = deg[order]
    for i in range(N_NODES):
        s_ = pos_slot[i]
        if degs_sorted[i] > cap[s_]:
            cap[s_] = degs_sorted[i]
    total = int(cap.sum())
    K = max(1, (total + P - 1) // P)
    return node_core, node_slot, cap, K


def _prepare(x, edge_index, edge_attr):
    src = np.asarray(edge_index[0])
    node_core, node_slot, cap, K = _partition_graph(src)
    rows_per_bin = K * P
    rows = BINS * rows_per_bin
    slot_base = np.concatenate([[0], np.cumsum(cap)])  # [129]

    # linear edge slot within bin: l = slot_base[s] + pos_within_node
    # dram row within bin block (e-major layout): (l % P) * K + (l // P)
    order = np.argsort(src, kind="stable")
    src_s = src[order]
    counts = np.bincount(src_s, minlength=N_NODES)
    starts = np.concatenate([[0], np.cumsum(counts)])[:-1]
    pos = np.arange(N_EDGES, dtype=np.int64) - starts[src_s]

    e_core = node_core[src_s].astype(np.int64)
    e_slotg = node_slot[src_s].astype(np.int64)
    e_bin = e_slotg // P
    e_s = e_slotg % P
    l = slot_base[e_s] + pos
    dest_row = e_bin * rows_per_bin + (l % P) * K + (l // P)

    edge_dt = ml_dtypes.bfloat16 if EDGE_BF16 else np.float32
    edges_all = np.zeros((NCORES, rows, D), edge_dt)
    edges_all[e_core, dest_row] = np.asarray(edge_attr)[order].astype(edge_dt)

    # constant one-hot operands: M[e, t, n] = (slot_of(t*P + e) == n)
    slot_of = np.searchsorted(slot_base, np.arange(rows_per_bin), side="right") - 1
    slot_of = np.minimum(slot_of, P - 1)
    M = np.zeros((P, K, P), edge_dt)
    lin = np.arange(rows_per_bin)
    M[lin % P, lin // P, slot_of] = 1.0
    # rows past the real capacity: zero attr anyway; keep mapping to slot 127

    xt_all = np.zeros((NCORES, D, NODES_PER_CORE), ml_dtypes.bfloat16)
    xt_all[node_core, :, node_slot] = np.asarray(x).astype(ml_dtypes.bfloat16)

    return {
        "node_core": node_core,
        "node_slot": node_slot,
        "K": K,
        "edges": edges_all,
        "onehot": np.ascontiguousarray(M.reshape(P, K * P)),
        "xt": xt_all,
    }


# ------------------------------------------------------------- device build

def _build(K):
    tot_tiles = BINS * K
    nc = bacc.Bacc("TRN2", debug=False, num_devices=NCORES)

    edges_d = nc.declare_dram_parameter("edges", [tot_tiles * P, D], BF16, isOutput=False)
    onehot_d = nc.declare_dram_parameter("onehot", [P, K * P], BF16, isOutput=False)
    xt_d = nc.declare_dram_parameter("xt", [D, NODES_PER_CORE], BF16, isOutput=False)
    w1a_d = nc.declare_dram_parameter("w1a", [D, D], BF16, isOutput=False)
    w1b_d = nc.declare_dram_parameter("w1b", [D, D], BF16, isOutput=False)
    w2_d = nc.declare_dram_parameter("w2", [D, D], BF16, isOutput=False)
    w3_d = nc.declare_dram_parameter("w3", [D, D], BF16, isOutput=False)
    b1_d = nc.declare_dram_parameter("b1", [D], F32, isOutput=False)
    b2_d = nc.declare_dram_parameter("b2", [D], F32, isOutput=False)
    b3_d = nc.declare_dram_parameter("b3", [D], F32, isOutput=False)
    g1_d = nc.declare_dram_parameter("g1", [D], F32, isOutput=False)
    g2_d = nc.declare_dram_parameter("g2", [D], F32, isOutput=False)
    be1_d = nc.declare_dram_parameter("be1", [D], F32, isOutput=False)
    be2_d = nc.declare_dram_parameter("be2", [D], F32, isOutput=False)
    out_d = nc.declare_dram_parameter("out", [D, NODES_PER_CORE], BF16, isOutput=True)
    dbg_d = nc.declare_dram_parameter("dbg", [NCORES], F32, isOutput=True)

    # collective bounce buffers
    warm_in = nc.dram_tensor("warm_in", [1], F32)
    warm_out = nc.dram_tensor("warm_out", [NCORES], F32, addr_space="Shared")
    cc_in = [nc.dram_tensor(f"cc_in{i}", [2 * D], F32) for i in range(2)]
    cc_out = [
        nc.dram_tensor(f"cc_out{i}", [NCORES * 2 * D], F32, addr_space="Shared")
        for i in range(2)
    ]
    rg = [list(range(NCORES))]

    edt = BF16 if EDGE_BF16 else F32
    inv_n = 1.0 / float(N_NODES)

    with (
        TileContext(nc) as tc,
        tc.tile_pool(name="const", bufs=1) as cpool,
        tc.tile_pool(name="big", bufs=1) as big,
        tc.tile_pool(name="ebin", bufs=6) as ebin_pool,
        tc.tile_pool(name="work", bufs=4) as work,
        tc.tile_pool(name="psum", bufs=4, space="PSUM") as pp,
        tc.tile_pool(name="psum_mlp", bufs=2, space="PSUM") as pmlp,
    ):
        # ---- constants
        oh_sb = cpool.tile([P, K, P], BF16)
        nc.sync.dma_start(out=oh_sb.rearrange("e t f -> e (t f)"), in_=onehot_d[:])

        def load_col(dram, nm):
            t = cpool.tile([P, 1], F32, name=f"col_{nm}")
            nc.sync.dma_start(out=t[:], in_=dram[:, None])
            return t

        b1_sb, b2_sb, b3_sb = load_col(b1_d, "b1"), load_col(b2_d, "b2"), load_col(b3_d, "b3")
        g1_sb, g2_sb = load_col(g1_d, "g1"), load_col(g2_d, "g2")
        be1_sb, be2_sb = load_col(be1_d, "be1"), load_col(be2_d, "be2")

        def load_w(dram, nm):
            t = cpool.tile([P, D], BF16, name=f"w_{nm}")
            nc.sync.dma_start(out=t[:], in_=dram[:])
            return t

        w1a_sb = load_w(w1a_d, "w1a")
        w1b_sb = load_w(w1b_d, "w1b")
        w2_sb = load_w(w2_d, "w2")
        w3_sb = load_w(w3_d, "w3")

        # ---- preload ACT LUT tables off the critical path
        lutw = cpool.tile([P, 1], F32, name="lutw")
        nc.scalar.activation(out=lutw[:], in_=b1_sb[:], func=mybir.ActivationFunctionType.Silu)
        nc.scalar.activation(out=lutw[:], in_=b1_sb[:], func=mybir.ActivationFunctionType.Identity, bias=b2_sb[:, :1])
        nc.scalar.sqrt(lutw[:], b1_sb[:])

        # ---- early dummy AllGather to absorb first-collective sync cost
        zz = cpool.tile([1, 1], F32)
        nc.vector.memset(zz[:], 0.0)
        nc.sync.dma_start(out=warm_in[:], in_=zz[0, :])
        nc.gpsimd.collective_compute(
            "AllGather", mybir.AluOpType.bypass, replica_groups=rg,
            ins=[warm_in[:]], outs=[warm_out[:]],
        )
        # ---- big activation buffers (feature-major)
        xt_bf = big.tile([P, NODES_PER_CORE], BF16)
        nc.sync.dma_start(out=xt_bf[:], in_=xt_d[:])
        agg_bf = big.tile([P, NODES_PER_CORE], BF16)
        h1_bf = big.tile([P, NODES_PER_CORE], BF16)
        h2_bf = big.tile([P, NODES_PER_CORE], BF16)

        # per-layer stat partials: [128, tiles] (sum, sumsq)
        s_part = [cpool.tile([P, MLP_TILES], F32, name=f"s_part{i}") for i in range(2)]
        q_part = [cpool.tile([P, MLP_TILES], F32, name=f"q_part{i}") for i in range(2)]

        def mlp_tile_cols(j):
            c0 = j * MLP_TILE
            c1 = min(NODES_PER_CORE, c0 + MLP_TILE)
            return c0, c1

        def silu_layer(li, j, ps, h_out, b_sb):
            """SiLU(ps + b) -> h_out cols of tile j, plus sum/sumsq partials
            (dummy slots excluded on the last tile)."""
            c0, c1 = mlp_tile_cols(j)
            w = c1 - c0
            real_w = w if j < MLP_TILES - 1 else (w - DUMMY_PER_CORE)
            nc.scalar.activation(
                out=h_out[:, c0 : c0 + real_w], in_=ps[:, :real_w],
                func=mybir.ActivationFunctionType.Silu,
                bias=b_sb[:, :1], scale=1.0,
                accum_out=s_part[li][:, j : j + 1],
            )
            if real_w < w:
                nc.scalar.activation(
                    out=h_out[:, c0 + real_w : c1], in_=ps[:, real_w:w],
                    func=mybir.ActivationFunctionType.Silu,
                    bias=b_sb[:, :1], scale=1.0,
                )
            sq = work.tile([P, MLP_TILE], BF16, tag="sq")
            nc.vector.scalar_tensor_tensor(
                out=sq[:, :real_w], in0=h_out[:, c0 : c0 + real_w], scalar=0.0,
                in1=h_out[:, c0 : c0 + real_w],
                op0=mybir.AluOpType.add, op1=mybir.AluOpType.mult,
                accum_out=q_part[li][:, j : j + 1],
            )

        # ---- edge phase with interleaved layer-1 MLP
        tile_idx = 0
        for g in range(BINS):
            kg = K
            ebin_c = ebin_pool.tile([P, kg, D], BF16, tag="ebin")
            nc.sync.dma_start(
                out=ebin_c.rearrange("e t f -> e (t f)"),
                in_=edges_d[tile_idx * P : (tile_idx + kg) * P, :].rearrange(
                    "(e q) f -> e (q f)", e=P
                ),
            )

            ps_agg = pp.tile([P, P], F32, tag="ps_agg")
            for t in range(kg):
                nc.tensor.matmul(
                    out=ps_agg[:], lhsT=ebin_c[:, t, :], rhs=oh_sb[:, t, :],
                    start=(t == 0), stop=(t == kg - 1),
                )
            tile_idx += kg
            nc.scalar.copy(agg_bf[:, g * P : (g + 1) * P], ps_agg[:])

            if g % 4 == 3 or g == BINS - 1:
                j = g // 4
                c0, c1 = mlp_tile_cols(j)
                ps1 = pmlp.tile([P, MLP_TILE], F32, tag="ps_mlp")
                nc.tensor.matmul(
                    out=ps1[:, : c1 - c0], lhsT=w1a_sb[:], rhs=xt_bf[:, c0:c1],
                    start=True, stop=False,
                )
                nc.tensor.matmul(
                    out=ps1[:, : c1 - c0], lhsT=w1b_sb[:], rhs=agg_bf[:, c0:c1],
                    start=False, stop=True,
                )
                silu_layer(0, j, ps1, h1_bf, b1_sb)

        # ---- BN stats + weight folding helper
        def bn_fold(li, w_sb, g_sb, be_sb, b_next_sb):
            """AllGather (sum, sumsq), compute scale/shift, fold into w_sb
            (rows) producing (w_fold, b_fold)."""
            s_loc = cpool.tile([P, 1], F32, tag=f"s_loc{li}")
            q_loc = cpool.tile([P, 1], F32, tag=f"q_loc{li}")
            nc.vector.reduce_sum(out=s_loc[:], in_=s_part[li][:], axis=mybir.AxisListType.X)
            nc.vector.reduce_sum(out=q_loc[:], in_=q_part[li][:], axis=mybir.AxisListType.X)
            nc.sync.dma_start(out=cc_in[li][0:D], in_=s_loc[:, 0])
            nc.sync.dma_start(out=cc_in[li][D : 2 * D], in_=q_loc[:, 0])
            nc.gpsimd.collective_compute(
                "AllGather", mybir.AluOpType.bypass, replica_groups=rg,
                ins=[cc_in[li][:]], outs=[cc_out[li][:]],
            )
            gath = work.tile([P, 2, NCORES], F32, tag=f"gath{li}")
            cc_view = cc_out[li].rearrange("(r kf) -> kf r", kf=2 * D)
            nc.sync.dma_start(out=gath[:, 0, :], in_=cc_view[0:D, :])
            nc.sync.dma_start(out=gath[:, 1, :], in_=cc_view[D : 2 * D, :])
            mean = cpool.tile([P, 1], F32, tag=f"mean{li}")
            msq = cpool.tile([P, 1], F32, tag=f"msq{li}")
            nc.vector.reduce_sum(out=mean[:], in_=gath[:, 0, :], axis=mybir.AxisListType.X)
            nc.vector.tensor_scalar_mul(mean[:], mean[:], inv_n)
            nc.vector.reduce_sum(out=msq[:], in_=gath[:, 1, :], axis=mybir.AxisListType.X)
            nc.vector.tensor_scalar_mul(msq[:], msq[:], inv_n)
            var = cpool.tile([P, 1], F32, tag=f"var{li}")
            nc.vector.tensor_tensor(
                out=var[:], in0=mean[:], in1=mean[:], op=mybir.AluOpType.mult
            )
            nc.vector.tensor_tensor(
                out=var[:], in0=msq[:], in1=var[:], op=mybir.AluOpType.subtract
            )
            nc.vector.tensor_scalar_add(var[:], var[:], BN_EPS)
            std = cpool.tile([P, 1], F32, tag=f"std{li}")
            nc.scalar.sqrt(std[:], var[:])
            rstd = cpool.tile([P, 1], F32, tag=f"rstd{li}")
            nc.vector.reciprocal(rstd[:], std[:])
            scale = cpool.tile([P, 1], F32, tag=f"scale{li}")
            nc.vector.tensor_tensor(
                out=scale[:], in0=g_sb[:], in1=rstd[:], op=mybir.AluOpType.mult
            )
            shift = cpool.tile([P, 1], F32, tag=f"shift{li}")
            nc.vector.tensor_tensor(
                out=shift[:], in0=mean[:], in1=scale[:], op=mybir.AluOpType.mult
            )
            nc.vector.tensor_tensor(
                out=shift[:], in0=be_sb[:], in1=shift[:], op=mybir.AluOpType.subtract
            )
            w_fold = cpool.tile([P, D], BF16, tag=f"wf{li}")
            nc.vector.tensor_scalar(
                out=w_fold[:], in0=w_sb[:], scalar1=scale[:, :1], scalar2=None,
                op0=mybir.AluOpType.mult,
            )
            shift_bf = cpool.tile([P, 1], BF16, tag=f"shift_bf{li}")
            nc.vector.tensor_copy(shift_bf[:], shift[:])
            ps_b = pmlp.tile([P, MLP_TILE], F32, tag="ps_mlp")
            nc.tensor.matmul(out=ps_b[:, :1], lhsT=w_sb[:], rhs=shift_bf[:], start=True, stop=True)
            b_fold = cpool.tile([P, 1], F32, tag=f"bf{li}")
            nc.vector.tensor_tensor(
                out=b_fold[:], in0=ps_b[:, :1], in1=b_next_sb[:], op=mybir.AluOpType.add
            )
            return w_fold, b_fold

        # ---- layer 2
        w2f, b2f = bn_fold(0, w2_sb, g1_sb, be1_sb, b2_sb)
        for j in range(MLP_TILES):
            c0, c1 = mlp_tile_cols(j)
            ps2 = pmlp.tile([P, MLP_TILE], F32, tag="ps_mlp")
            nc.tensor.matmul(
                out=ps2[:, : c1 - c0], lhsT=w2f[:], rhs=h1_bf[:, c0:c1],
                start=True, stop=True,
            )
            silu_layer(1, j, ps2, h2_bf, b2f)

        # ---- layer 3
        w3f, b3f = bn_fold(1, w3_sb, g2_sb, be2_sb, b3_sb)
        for j in range(MLP_TILES):
            c0, c1 = mlp_tile_cols(j)
            ps3 = pmlp.tile([P, MLP_TILE], F32, tag="ps_mlp")
            nc.tensor.matmul(
                out=ps3[:, : c1 - c0], lhsT=w3f[:], rhs=h2_bf[:, c0:c1],
                start=True, stop=True,
            )
            o = work.tile([P, MLP_TILE], BF16, tag="otile")
            if j % 2 == 0:
                nc.vector.tensor_scalar(
                    out=o[:, : c1 - c0], in0=ps3[:, : c1 - c0],
                    scalar1=b3f[:, :1], scalar2=None, op0=mybir.AluOpType.add,
                )
            else:
                nc.scalar.activation(
                    out=o[:, : c1 - c0], in_=ps3[:, : c1 - c0],
                    func=mybir.ActivationFunctionType.Identity,
                    bias=b3f[:, :1], scale=1.0,
                )
            nc.sync.dma_start(out=out_d[:, c0:c1], in_=o[:, : c1 - c0])

        nc.sync.dma_start(out=dbg_d[:], in_=warm_out[:])

    nc.finalize()
    return nc


# ------------------------------------------------------------------- driver

def kernel(x, edge_index, edge_attr, W1, b1, g1, be1, W2, b2, g2, be2, W3, b3):
    global LAST_EXEC_NS
    x = np.asarray(x)
    prep = _prepare(x, np.asarray(edge_index), np.asarray(edge_attr))
    key = prep["K"]
    if key not in _CACHE:
        _CACHE[key] = _build(prep["K"])
    nc = _CACHE[key]

    W1 = np.asarray(W1, np.float32)
    bf = ml_dtypes.bfloat16
    shared = {
        "w1a": np.ascontiguousarray(W1[:D]).astype(bf),
        "w1b": np.ascontiguousarray(W1[D:]).astype(bf),
        "w2": np.asarray(W2, np.float32).astype(bf),
        "w3": np.asarray(W3, np.float32).astype(bf),
        "b1": np.asarray(b1, np.float32), "b2": np.asarray(b2, np.float32),
        "b3": np.asarray(b3, np.float32),
        "g1": np.asarray(g1, np.float32), "g2": np.asarray(g2, np.float32),
        "be1": np.asarray(be1, np.float32), "be2": np.asarray(be2, np.float32),
    }
    in_maps = []
    for c in range(NCORES):
        m = dict(shared)
        m["edges"] = prep["edges"][c]
        m["onehot"] = prep["onehot"]
        m["xt"] = prep["xt"][c]
        in_maps.append(m)

    trace = bool(os.environ.get("KERNEL_TRACE"))
    res = run_bass_kernel_spmd(
        nc, in_maps, core_ids=list(range(NCORES)), trace=trace
    )
    LAST_EXEC_NS = res.exec_time_ns

    outs = np.stack([np.asarray(res.results[c]["out"]) for c in range(NCORES)])
    # [core, D, slot] -> [N, D]
    out = outs[prep["node_core"], :, prep["node_slot"]]
    return np.ascontiguousarray(out.astype(np.float32))
